# revision 9
# baseline (speedup 1.0000x reference)
"""Trainium2 Bass kernel for a 2-layer GCN (GCNConv x2 + MLP head),
8-core SPMD via run_bass_kernel_spmd.

v4 design notes (what profiling showed):
- The aggregation gather is bound by one-packet-in-flight per
  (SDMA engine, SWDGE queue) x HBM latency ~= 0.38 rows/ns with 4
  queues; the sweeps already run at that floor, so v4 hides everything
  else behind the gather stream:
  * AllGather chunks are bank-aligned (49-block chunk = banks 0-1), so
    each sweep's bank-0/1 gathers are gated only on chunk 1 and drain
    while chunk 2 is still on the wire.
  * Phase A makes W1 the stationary matmul operand (5 LDWEIGHTS per 512
    nodes instead of 5 per 128) with a [128, 512] PSUM tile, scales by
    dinv on the DVE, transposes 128x128 blocks on the DVE, and stores
    4-block slabs.
- Self-loops are applied by one identity matmul per dst block from the
  local table slice (keeps the div-banked gather segments balanced).
- Gather segments are div-banked, slots sorted by source row, pad slots
  duplicate the segment's last real row (page-hit re-reads).
"""

import numpy as np

import concourse.bacc as bacc
import concourse.mybir as mybir
import concourse.tile as tile

F32 = mybir.dt.float32
BF16 = mybir.dt.bfloat16
I16 = mybir.dt.int16

N, IN_DIM, HID, MID, OUT = 100000, 518, 128, 64, 4
NCORES = 8
NPC = N // NCORES            # 12500
NB = (NPC + 127) // 128      # 98
NPAD = NB * 128              # 12544
R1 = 49 * 128                # 6272 rows in AllGather chunk 1 (49 blocks)
R2 = NPAD - R1               # 6272 rows in chunk 2 (49 blocks)
NPHYS = NCORES * NPAD        # 100352 physical table rows
NBANK = 4
BR = NPHYS // NBANK          # 25088 rows per bank (int16-indexable)
# chunk 1 rows occupy exactly banks 0-1
assert NCORES * R1 == 2 * BR
G = 8                        # dst blocks per gather group
NG = (NB + G - 1) // G       # 13
B1_GROUPS = 7                # groups 0-6 = blocks 0-55 (covers rows < R1)
KT = [(k, min(128, IN_DIM - k)) for k in range(0, IN_DIM, 128)]


def default_cfg():
    return dict(N=N, NCORES=NCORES, IN_DIM=IN_DIM, HID=HID, MID=MID, OUT=OUT)


def derive(cfg):
    cfg = dict(cfg)
    cfg["NPC"] = NPC
    cfg["NB"] = NB
    return cfg


def _group_blocks(g):
    return range(g * G, min((g + 1) * G, NB))


def _phys_of(node):
    c = node // NPC
    r = node % NPC
    return np.where(r < R1, c * R1 + r, NCORES * R1 + c * R2 + (r - R1))


def make_plan(edge_index):
    """Host-side graph preprocessing."""
    src = np.asarray(edge_index[0], dtype=np.int64)
    dst = np.asarray(edge_index[1], dtype=np.int64)
    # self-loops are applied on-chip via an identity matmul per block;
    # only real edges go through the gather. Degrees still count A + I.
    deg = (np.bincount(dst, minlength=N) + 1).astype(np.float64)
    dinv = 1.0 / np.sqrt(deg)

    order = np.argsort(dst, kind="stable")
    ss, dd = src[order], dst[order]
    bounds = np.searchsorted(dd, np.arange(NCORES + 1) * NPC)

    phys = _phys_of(ss)
    bank_a = phys // BR
    inb_a = (phys % BR).astype(np.int64)

    percore = []
    cnts = np.zeros((NCORES, NB * NBANK), dtype=np.int64)
    for c in range(NCORES):
        lo, hi = bounds[c], bounds[c + 1]
        dloc = dd[lo:hi] - c * NPC
        blk = dloc >> 7
        key = blk * NBANK + bank_a[lo:hi]
        cnts[c] = np.bincount(key, minlength=NB * NBANK)
        percore.append((lo, hi, dloc, key))

    nch_bq = np.ceil(cnts.max(axis=0) / 128).astype(np.int64)
    nch_bq_2d = nch_bq.reshape(NB, NBANK)
    n_b = nch_bq_2d.sum(axis=1)
    chunk_base_b = np.zeros(NB + 1, dtype=np.int64)
    chunk_base_b[1:] = np.cumsum(n_b)
    TCH = int(chunk_base_b[-1])

    S_gq = np.zeros((NG, NBANK), dtype=np.int64)
    seg_choff = np.zeros((NB, NBANK), dtype=np.int64)
    for g in range(NG):
        for q in range(NBANK):
            off = 0
            for b in _group_blocks(g):
                seg_choff[b, q] = off
                off += nch_bq_2d[b, q]
            S_gq[g, q] = off * 128
    flat = S_gq.reshape(-1)
    starts = np.zeros(NG * NBANK, dtype=np.int64)
    starts[1:] = np.cumsum(flat)[:-1]
    TOTSLOT = int(flat.sum())

    qoff_bq = np.zeros((NB, NBANK), dtype=np.int64)
    qoff_bq[:, 1:] = np.cumsum(nch_bq_2d, axis=1)[:, :-1]
    gof = np.zeros((NB, NBANK), dtype=np.int64)
    for b in range(NB):
        g = b // G
        for q in range(NBANK):
            gof[b, q] = starts[g * NBANK + q] + seg_choff[b, q] * 128

    segid = np.zeros(TOTSLOT, dtype=np.int64)
    for b in range(NB):
        for q in range(NBANK):
            s0 = gof[b, q]
            segid[s0:s0 + nch_bq_2d[b, q] * 128] = b * NBANK + q

    idx16 = np.zeros((NCORES, 128, TOTSLOT // 16), dtype=np.int16)
    bf = mybir.dt.np(BF16)
    dstL = np.full((NCORES, 128, TCH), -1.0, dtype=np.float32)
    for c in range(NCORES):
        lo, hi, dloc, key = percore[c]
        inb = inb_a[lo:hi]
        # sort by (segment, src row): the gather walks each bank region
        # in ascending order
        order2 = np.lexsort((inb, key))
        kk = key[order2]
        ii = inb[order2].astype(np.int16)
        seg_starts = np.zeros(NB * NBANK, dtype=np.int64)
        seg_starts[1:] = np.cumsum(cnts[c])[:-1]
        pos = np.arange(hi - lo) - seg_starts[kk]
        b2, q2 = kk // NBANK, kk % NBANK
        j = pos >> 7
        p = pos & 127
        t = chunk_base_b[b2] + qoff_bq[b2, q2] + j
        dstL[c, p, t] = (dloc[order2] & 127).astype(np.float32)
        s_glob = gof[b2, q2] + pos

        slotv = np.zeros(TOTSLOT, dtype=np.int16)
        slotv[s_glob] = ii
        lastv = np.zeros(NB * NBANK, dtype=np.int16)
        lastv[kk] = ii
        padmask = np.ones(TOTSLOT, dtype=bool)
        padmask[s_glob] = False
        slotv[padmask] = lastv[segid[padmask]]

        sv = slotv.reshape(TOTSLOT // 16, 16).T
        for r in range(8):
            idx16[c, 16 * r:16 * r + 16, :] = sv

    dinv2C = np.zeros((NCORES, 128, NB), dtype=np.float32)
    dinvBT = np.zeros((NCORES, 128, NPAD), dtype=bf)
    rdinvR = np.zeros((NCORES, 1, NPAD), dtype=bf)
    for c in range(NCORES):
        dv = np.zeros(NPAD, dtype=np.float32)
        dv[:NPC] = dinv[c * NPC:(c + 1) * NPC]
        dinv2C[c] = (dv * dv).reshape(NB, 128).T
        dinvBT[c] = np.broadcast_to(dv.astype(bf), (128, NPAD))
        rv = np.zeros(NPAD, dtype=np.float32)
        rv[:NPC] = np.sqrt(deg[c * NPC:(c + 1) * NPC])
        rdinvR[c] = rv.astype(bf)[None, :]

    NBMAX = int(n_b.max())
    iotaB = np.broadcast_to(
        np.arange(128, dtype=np.float32), (128, NBMAX, 128)).astype(bf)
    ident = np.eye(128, dtype=np.float32).astype(bf)
    return dict(ident=ident, TCH=TCH, TOTSLOT=TOTSLOT, NBMAX=NBMAX,
                nch_bq=nch_bq_2d, n_b=n_b, chunk_base_b=chunk_base_b,
                qoff_bq=qoff_bq, seg_choff=seg_choff, S_gq=S_gq,
                starts=starts, idx16=idx16, dstL=dstL.astype(bf),
                iotaB=iotaB, dinv2C=dinv2C, dinvBT=dinvBT, rdinvR=rdinvR)


def build_gcn(plan):
    TCH, TOTSLOT, NBMAX = plan["TCH"], plan["TOTSLOT"], plan["NBMAX"]
    nch_bq = plan["nch_bq"]
    n_b = plan["n_b"]
    chunk_base_b = plan["chunk_base_b"]
    qoff_bq = plan["qoff_bq"]
    seg_choff = plan["seg_choff"]
    S_gq = plan["S_gq"]
    starts = plan["starts"]

    nc = bacc.Bacc(None, num_swdge_queues=NBANK,
                   dynamic_dma_scratch_size=32768)
    xT = nc.dram_tensor("xT", [IN_DIM, NPAD], BF16, kind="ExternalInput")
    idx16 = nc.dram_tensor("idx16", [128, TOTSLOT // 16], I16,
                           kind="ExternalInput")
    dstL = nc.dram_tensor("dstL", [128, TCH], BF16, kind="ExternalInput")
    iotaB = nc.dram_tensor("iotaB", [128, NBMAX, 128], BF16,
                           kind="ExternalInput")
    ident = nc.dram_tensor("ident", [128, 128], BF16, kind="ExternalInput")
    w1 = nc.dram_tensor("w1", [IN_DIM, HID], BF16, kind="ExternalInput")
    w2 = nc.dram_tensor("w2", [HID, HID], BF16, kind="ExternalInput")
    wm1 = nc.dram_tensor("wm1", [HID, MID], BF16, kind="ExternalInput")
    wm2 = nc.dram_tensor("wm2", [MID, OUT], BF16, kind="ExternalInput")
    b1r = nc.dram_tensor("b1r", [1, HID], BF16, kind="ExternalInput")
    b2c = nc.dram_tensor("b2c", [HID, 1], F32, kind="ExternalInput")
    bm1c = nc.dram_tensor("bm1c", [MID, 1], F32, kind="ExternalInput")
    bm2c = nc.dram_tensor("bm2c", [OUT, 1], F32, kind="ExternalInput")
    dinv2C = nc.dram_tensor("dinv2C", [128, NB], F32, kind="ExternalInput")
    dinvBT = nc.dram_tensor("dinvBT", [128, NPAD], BF16,
                            kind="ExternalInput")
    rdinvR = nc.dram_tensor("rdinvR", [1, NPAD], BF16, kind="ExternalInput")
    outT = nc.dram_tensor("outT", [OUT, NPAD], F32, kind="ExternalOutput")

    m0_loc = nc.dram_tensor("m0_loc", [NPAD, HID], BF16)
    m0_full = nc.dram_tensor("m0_full", [NPHYS, HID], BF16,
                             addr_space="Shared")
    m1_loc = nc.dram_tensor("m1_loc", [NPAD, HID], BF16)
    m1_full = nc.dram_tensor("m1_full", [NPHYS, HID], BF16,
                             addr_space="Shared")
    cc_sem = nc.alloc_semaphore(name="cc_sem")

    # -------- phase A: table rows T1 = dinv * (x @ W1) (node slice) -----
    def phase_a(b0, b1):
        r0 = b0 * 128
        width = (b1 - b0) * 128
        with tile.TileContext(nc) as tc:
            with (
                tc.tile_pool(name="w1p", bufs=1) as w1p,
                tc.tile_pool(name="xtp", bufs=1) as xtp,
                tc.tile_pool(name="hmp", bufs=2) as hmp,
                tc.tile_pool(name="h0p", bufs=2) as h0p,
                tc.tile_pool(name="psA", bufs=2, space="PSUM") as psA,
                tc.tile_pool(name="psAT", bufs=3, space="PSUM") as psAT,
            ):
                w1_sb = []
                for k0, ksz in KT:
                    t = w1p.tile([ksz, HID], BF16, tag=f"w1_{k0}")
                    nc.sync.dma_start(out=t[:], in_=w1[k0:k0 + ksz, :])
                    w1_sb.append(t)
                dinv_sb = w1p.tile([128, width], BF16, tag="dinvA")
                nc.sync.dma_start(out=dinv_sb[:],
                                  in_=dinvBT[:, r0:r0 + width])
                identA_sb = w1p.tile([128, 128], BF16, tag="identA")
                nc.sync.dma_start(out=identA_sb[:], in_=ident[:])
                xts = []
                for ti, (k0, ksz) in enumerate(KT):
                    t = xtp.tile([ksz, width], BF16, tag=f"xt_{ti}")
                    nc.sync.dma_start(out=t[:], in_=xT[k0:k0 + ksz,
                                                      r0:r0 + width])
                    xts.append(t)
                for n0 in range(0, width, 512):
                    W = min(512, width - n0)
                    nj = W // 128
                    ps = psA.tile([128, 512], F32, space="PSUM")
                    for ti in range(len(KT)):
                        nc.tensor.matmul(
                            out=ps[:, :W],
                            lhsT=w1_sb[ti][:],
                            rhs=xts[ti][:, n0:n0 + W],
                            start=(ti == 0), stop=(ti == len(KT) - 1),
                        )
                    hm = hmp.tile([128, 512], BF16, tag="hm")
                    nc.vector.tensor_tensor(
                        out=hm[:, :W], in0=ps[:, :W],
                        in1=dinv_sb[:, n0:n0 + W],
                        op=mybir.AluOpType.mult)
                    slab = h0p.tile([128, 4, HID], BF16, tag="h0slab")
                    for j in range(nj):
                        pst = psAT.tile([128, 128], BF16, space="PSUM",
                                        tag="pst", name="pst")
                        nc.tensor.transpose(
                            out=pst[:, :],
                            in_=hm[:, j * 128:(j + 1) * 128],
                            identity=identA_sb[:])
                        nc.scalar.activation(
                            out=slab[:, j, :], in_=pst[:],
                            func=mybir.ActivationFunctionType.Copy)
                    nc.sync.dma_start(
                        out=m0_loc.ap()[r0 + n0:r0 + n0 + W, :].rearrange(
                            "(j p) f -> p j f", j=nj),
                        in_=slab[:, :nj, :])

    phase_a(0, 49)
    nc.gpsimd.collective_compute(
        "AllGather", mybir.AluOpType.bypass,
        replica_groups=[list(range(NCORES))],
        ins=[m0_loc.ap()[0:R1, :].opt()],
        outs=[m0_full.ap()[0:NCORES * R1, :].opt()],
    ).then_inc(cc_sem, 1)
    phase_a(49, NB)
    nc.gpsimd.collective_compute(
        "AllGather", mybir.AluOpType.bypass,
        replica_groups=[list(range(NCORES))],
        ins=[m0_loc.ap()[R1:NPAD, :].opt()],
        outs=[m0_full.ap()[NCORES * R1:NPHYS, :].opt()],
    ).then_inc(cc_sem, 1)

    def gather_instr(gqp, idx_sb, cb0, table, g, q):
        slots = int(S_gq[g, q])
        t = gqp.tile([128, slots // 128, HID], BF16, tag=f"g{q}",
                     name=f"g{q}_{g}")
        base = int(starts[g * NBANK + q])
        nc.gpsimd.dma_gather(
            out_ap=t[:, :, :],
            in_ap=table.ap()[q * BR:(q + 1) * BR, :],
            idxs_ap=idx_sb[:, (base - cb0) // 16:
                           (base + slots - cb0) // 16],
            num_idxs=slots,
            num_idxs_reg=slots,
            elem_size=HID,
            elem_step=HID,
            single_packet=False,
            queue_num=q,
        )
        return t

    # ---------------- phase B: layer-1 aggregation + @W2 ----------------
    def sweep_b(glo, ghi, sem_hi):
        cb0 = int(starts[glo * NBANK])
        cb1 = (int(starts[(ghi - 1) * NBANK + NBANK - 1])
               + int(S_gq[ghi - 1, NBANK - 1]))
        t_lo = int(chunk_base_b[glo * G])
        t_hi = int(chunk_base_b[min(ghi * G, NB)])
        with tile.TileContext(nc) as tc:
            with (
                tc.tile_pool(name="metaB", bufs=1) as meta,
                tc.tile_pool(name="gB", bufs=2) as gqp,
                tc.tile_pool(name="eB", bufs=3) as eP,
                tc.tile_pool(name="psB", bufs=4, space="PSUM") as psAcc,
                tc.tile_pool(name="psB2", bufs=2, space="PSUM") as psT,
                tc.tile_pool(name="hB", bufs=3) as hP,
                tc.tile_pool(name="wB", bufs=1) as wP,
            ):
                idx_sb = meta.tile([128, (cb1 - cb0) // 16], I16)
                nc.sync.dma_start(out=idx_sb[:],
                                  in_=idx16[:, cb0 // 16:cb1 // 16])
                dstL_sb = meta.tile([128, t_hi - t_lo], BF16)
                nc.sync.dma_start(out=dstL_sb[:], in_=dstL[:, t_lo:t_hi])
                iota_sb = meta.tile([128, NBMAX, 128], BF16)
                nc.sync.dma_start(out=iota_sb[:], in_=iotaB[:])
                ident_sb = meta.tile([128, 128], BF16)
                nc.sync.dma_start(out=ident_sb[:], in_=ident[:])
                w2_sb = wP.tile([HID, HID], BF16, tag="w2")
                nc.sync.dma_start(out=w2_sb[:], in_=w2[:])
                b1r_sb = wP.tile([1, HID], BF16, tag="b1r")
                nc.sync.dma_start(out=b1r_sb[:], in_=b1r[:])
                rdinv_sb = wP.tile([1, NPAD], BF16, tag="rdinv")
                nc.sync.dma_start(out=rdinv_sb[:], in_=rdinvR[:])
                dinv2C_sb = wP.tile([128, NB], F32, tag="dinv2C")
                nc.sync.dma_start(out=dinv2C_sb[:], in_=dinv2C[:])

                gt = {}
                slab = [None]
                for g in range(glo, ghi):
                    nblk = len(list(_group_blocks(g)))
                    mloc = hP.tile([128, G, HID], BF16, tag="mloc",
                                   name="mloc")
                    r0g = g * G * 128
                    nc.sync.dma_start(
                        out=mloc[:, :nblk, :],
                        in_=m0_loc.ap()[r0g:r0g + nblk * 128, :]
                        .rearrange("(j p) f -> p j f", j=nblk))
                    for q in range(NBANK):
                        gt[g, q] = gather_instr(gqp, idx_sb, cb0, m0_full,
                                                g, q)
                    for bi, b in enumerate(_group_blocks(g)):
                        nb = int(n_b[b])
                        t0 = int(chunk_base_b[b]) - t_lo
                        eq = eP.tile([128, NBMAX, 128], BF16, tag="eq",
                                     name="eq")
                        nc.vector.tensor_tensor(
                            out=eq[:, :nb, :], in0=iota_sb[:, :nb, :],
                            in1=dstL_sb[:, t0:t0 + nb].unsqueeze(2)
                            .broadcast_to([128, nb, 128]),
                            op=mybir.AluOpType.is_equal)
                        acc = psAcc.tile([HID, 128], F32, space="PSUM")
                        # self-loop: acc[h, d] += table[d, h]
                        nc.tensor.matmul(
                            out=acc[:], lhsT=mloc[:, bi, :],
                            rhs=ident_sb[:], start=True, stop=False)
                        for q in range(NBANK):
                            for j in range(int(nch_bq[b, q])):
                                nc.tensor.matmul(
                                    out=acc[:],
                                    lhsT=gt[g, q][:, int(seg_choff[b, q])
                                                  + j, :],
                                    rhs=eq[:, int(qoff_bq[b, q]) + j, :],
                                    start=False, stop=False,
                                )
                        # += b1 (x) rdinv closes the accumulation
                        nc.tensor.matmul(
                            out=acc[:], lhsT=b1r_sb[:],
                            rhs=rdinv_sb[:, b * 128:(b + 1) * 128],
                            start=False, stop=True)
                        v = hP.tile([HID, 128], BF16, tag="v", name="v")
                        nc.scalar.activation(
                            out=v[:], in_=acc[:],
                            func=mybir.ActivationFunctionType.Copy)
                        m1ps = psT.tile([128, HID], F32, space="PSUM")
                        nc.tensor.matmul(out=m1ps[:], lhsT=v[:],
                                         rhs=w2_sb[:],
                                         start=True, stop=True)
                        jj = b % 4
                        if jj == 0:
                            slab[0] = hP.tile([128, 4, HID], BF16,
                                              tag="m1slab", name="m1slab")
                        nc.scalar.activation(
                            out=slab[0][:, jj, :], in_=m1ps[:],
                            func=mybir.ActivationFunctionType.Copy,
                            scale=dinv2C_sb[:, b:b + 1])
                        if jj == 3 or b == NB - 1:
                            rows = (jj + 1) * 128
                            base2 = (b - jj) * 128
                            nc.sync.dma_start(
                                out=m1_loc.ap()[base2:base2 + rows, :]
                                .rearrange("(j p) f -> p j f", j=jj + 1),
                                in_=slab[0][:, :jj + 1, :])

    nc.gpsimd.wait_ge(cc_sem, 2)
    sweep_b(0, B1_GROUPS, 2)
    nc.gpsimd.collective_compute(
        "AllGather", mybir.AluOpType.bypass,
        replica_groups=[list(range(NCORES))],
        ins=[m1_loc.ap()[0:R1, :].opt()],
        outs=[m1_full.ap()[0:NCORES * R1, :].opt()],
    ).then_inc(cc_sem, 1)   # -> 3
    sweep_b(B1_GROUPS, NG, 2)
    nc.gpsimd.collective_compute(
        "AllGather", mybir.AluOpType.bypass,
        replica_groups=[list(range(NCORES))],
        ins=[m1_loc.ap()[R1:NPAD, :].opt()],
        outs=[m1_full.ap()[NCORES * R1:NPHYS, :].opt()],
    ).then_inc(cc_sem, 1)   # -> 4

    # ---------------- phase C: layer-2 aggregation + MLP head ------------
    nc.gpsimd.wait_ge(cc_sem, 4)
    with tile.TileContext(nc) as tc:
        with (
            tc.tile_pool(name="metaC", bufs=1) as meta,
            tc.tile_pool(name="gC", bufs=2) as gqp,
            tc.tile_pool(name="eC", bufs=3) as eP,
            tc.tile_pool(name="psC", bufs=4, space="PSUM") as psAcc,
            tc.tile_pool(name="psC2", bufs=2, space="PSUM") as psC2,
            tc.tile_pool(name="psC3", bufs=2, space="PSUM") as psC3,
            tc.tile_pool(name="hC", bufs=3) as hP,
            tc.tile_pool(name="wC", bufs=1) as wP,
        ):
            idx_sb = meta.tile([128, TOTSLOT // 16], I16)
            nc.sync.dma_start(out=idx_sb[:], in_=idx16[:])
            dstL_sb = meta.tile([128, TCH], BF16)
            nc.sync.dma_start(out=dstL_sb[:], in_=dstL[:])
            iota_sb = meta.tile([128, NBMAX, 128], BF16)
            nc.sync.dma_start(out=iota_sb[:], in_=iotaB[:])
            ident_sb = meta.tile([128, 128], BF16)
            nc.sync.dma_start(out=ident_sb[:], in_=ident[:])
            wm1_sb = wP.tile([HID, MID], BF16, tag="wm1")
            nc.sync.dma_start(out=wm1_sb[:], in_=wm1[:])
            wm2_sb = wP.tile([MID, OUT], BF16, tag="wm2")
            nc.sync.dma_start(out=wm2_sb[:], in_=wm2[:])
            b2_sb = wP.tile([HID, 1], F32, tag="b2")
            nc.sync.dma_start(out=b2_sb[:], in_=b2c[:])
            bm1_sb = wP.tile([MID, 1], F32, tag="bm1")
            nc.sync.dma_start(out=bm1_sb[:], in_=bm1c[:])
            bm2_sb = wP.tile([OUT, 1], F32, tag="bm2")
            nc.sync.dma_start(out=bm2_sb[:], in_=bm2c[:])
            dinv_sb = wP.tile([128, NPAD], BF16, tag="dinvBT")
            nc.sync.dma_start(out=dinv_sb[:], in_=dinvBT[:])

            gt = {}
            zslab = [None]
            for g in range(NG):
                nblk = len(list(_group_blocks(g)))
                mloc = hP.tile([128, G, HID], BF16, tag="mloc",
                               name="mloc")
                r0g = g * G * 128
                nc.sync.dma_start(
                    out=mloc[:, :nblk, :],
                    in_=m1_loc.ap()[r0g:r0g + nblk * 128, :].rearrange(
                        "(j p) f -> p j f", j=nblk))
                for q in range(NBANK):
                    gt[g, q] = gather_instr(gqp, idx_sb, 0, m1_full, g, q)
                zslab[0] = hP.tile([OUT, G, 128], F32, tag="zslab",
                                   name="zslab")
                for bi, b in enumerate(_group_blocks(g)):
                    nb = int(n_b[b])
                    t0 = int(chunk_base_b[b])
                    eq = eP.tile([128, NBMAX, 128], BF16, tag="eq",
                                 name="eq")
                    nc.vector.tensor_tensor(
                        out=eq[:, :nb, :], in0=iota_sb[:, :nb, :],
                        in1=dstL_sb[:, t0:t0 + nb].unsqueeze(2)
                        .broadcast_to([128, nb, 128]),
                        op=mybir.AluOpType.is_equal)
                    acc = psAcc.tile([HID, 128], F32, space="PSUM")
                    # self-loop: acc[h, d] += table[d, h]
                    nc.tensor.matmul(
                        out=acc[:], lhsT=mloc[:, bi, :], rhs=ident_sb[:],
                        start=True, stop=False)
                    mm = 0
                    for q in range(NBANK):
                        for j in range(int(nch_bq[b, q])):
                            mm += 1
                            nc.tensor.matmul(
                                out=acc[:],
                                lhsT=gt[g, q][:, int(seg_choff[b, q]) + j, :],
                                rhs=eq[:, int(qoff_bq[b, q]) + j, :],
                                start=False, stop=(mm == nb),
                            )
                    # w = dinv * acc ; r2T = Relu(w + b2)
                    w = hP.tile([HID, 128], BF16, tag="w", name="w")
                    nc.vector.scalar_tensor_tensor(
                        out=w[:], in0=acc[:], scalar=0.0,
                        in1=dinv_sb[:, b * 128:(b + 1) * 128],
                        op0=mybir.AluOpType.add,
                        op1=mybir.AluOpType.mult)
                    r2T = hP.tile([HID, 128], BF16, tag="r2T", name="r2T")
                    nc.scalar.activation(
                        out=r2T[:], in_=w[:],
                        func=mybir.ActivationFunctionType.Relu,
                        bias=b2_sb[:])
                    y1 = psC2.tile([MID, 128], F32, space="PSUM")
                    nc.tensor.matmul(out=y1[:], lhsT=wm1_sb[:], rhs=r2T[:],
                                     start=True, stop=True)
                    r1T = hP.tile([MID, 128], BF16, tag="r1T", name="r1T")
                    nc.scalar.activation(
                        out=r1T[:], in_=y1[:],
                        func=mybir.ActivationFunctionType.Relu,
                        bias=bm1_sb[:])
                    z = psC3.tile([OUT, 128], F32, space="PSUM")
                    nc.tensor.matmul(out=z[:], lhsT=wm2_sb[:], rhs=r1T[:],
                                     start=True, stop=True)
                    nc.vector.tensor_scalar(
                        out=zslab[0][:, bi, :], in0=z[:], scalar1=bm2_sb[:],
                        scalar2=None, op0=mybir.AluOpType.add)
                c0 = g * G * 128
                cols = nblk * 128
                nc.sync.dma_start(
                    out=outT.ap()[:, c0:c0 + cols].rearrange(
                        "o (j p) -> o j p", j=nblk),
                    in_=zslab[0][:, :nblk, :])

    nc.compile()
    return nc


def make_inmaps(plan, inputs):
    bf = mybir.dt.np(BF16)
    x = np.asarray(inputs["x"], dtype=np.float32)
    w1 = np.asarray(inputs["w1"], np.float32).astype(bf)
    w2 = np.asarray(inputs["w2"], np.float32).astype(bf)
    wm1 = np.asarray(inputs["wm1"], np.float32).astype(bf)
    wm2 = np.asarray(inputs["wm2"], np.float32).astype(bf)
    b1r = np.asarray(inputs["b1"], np.float32).astype(bf)[None, :]
    b2c = np.asarray(inputs["b2"], np.float32)[:, None]
    bm1c = np.asarray(inputs["bm1"], np.float32)[:, None]
    bm2c = np.asarray(inputs["bm2"], np.float32)[:, None]
    in_maps = []
    for c in range(NCORES):
        xTc = np.zeros((IN_DIM, NPAD), dtype=bf)
        xTc[:, :NPC] = x[c * NPC:(c + 1) * NPC].T.astype(bf)
        in_maps.append({
            "xT": xTc, "idx16": plan["idx16"][c],
            "dstL": plan["dstL"][c], "iotaB": plan["iotaB"],
            "ident": plan["ident"],
            "w1": w1, "w2": w2, "wm1": wm1, "wm2": wm2,
            "b1r": b1r, "b2c": b2c, "bm1c": bm1c, "bm2c": bm2c,
            "dinv2C": plan["dinv2C"][c],
            "dinvBT": plan["dinvBT"][c], "rdinvR": plan["rdinvR"][c],
        })
    return in_maps


def build(inputs):
    plan = make_plan(np.asarray(inputs["edge_index"]))
    nc = build_gcn(plan)
    in_maps = make_inmaps(plan, inputs)
    return nc, in_maps


def assemble(results, cfg=None):
    return np.concatenate(
        [np.asarray(results[c]["outT"], dtype=np.float32).T[:NPC]
         for c in range(NCORES)], axis=0)


def kernel(**inputs):
    """Full-input entry point: returns [N, 4] float32."""
    nc, in_maps = build(inputs)
    from concourse.bass_utils import run_bass_kernel_spmd
    res = run_bass_kernel_spmd(nc, in_maps, core_ids=list(range(NCORES)))
    return assemble(res.results)


# revision 13
# speedup vs baseline: 1.0635x; 1.0635x over previous
"""Trainium2 Bass kernel for a 2-layer GCN (GCNConv x2 + MLP head),
8-core SPMD via run_bass_kernel_spmd.

v4 design notes (what profiling showed):
- The aggregation gather is bound by one-packet-in-flight per
  (SDMA engine, SWDGE queue) x HBM latency ~= 0.38 rows/ns with 4
  queues; the sweeps already run at that floor, so v4 hides everything
  else behind the gather stream:
  * AllGather chunks are bank-aligned (49-block chunk = banks 0-1), so
    each sweep's bank-0/1 gathers are gated only on chunk 1 and drain
    while chunk 2 is still on the wire.
  * Phase A makes W1 the stationary matmul operand (5 LDWEIGHTS per 512
    nodes instead of 5 per 128) with a [128, 512] PSUM tile, scales by
    dinv on the DVE, transposes 128x128 blocks on the DVE, and stores
    4-block slabs.
- Self-loops are applied by one identity matmul per dst block from the
  local table slice (keeps the div-banked gather segments balanced).
- Gather segments are div-banked, slots sorted by source row, pad slots
  duplicate the segment's last real row (page-hit re-reads).
"""

import numpy as np

import concourse.bacc as bacc
import concourse.mybir as mybir
import concourse.tile as tile

F32 = mybir.dt.float32
BF16 = mybir.dt.bfloat16
I16 = mybir.dt.int16

N, IN_DIM, HID, MID, OUT = 100000, 518, 128, 64, 4
NCORES = 8
NPC = N // NCORES            # 12500
NB = (NPC + 127) // 128      # 98
NPAD = NB * 128              # 12544
R1 = 48 * 128                # 6144 rows in AllGather chunk 1 (48 blocks)
R2 = NPAD - R1               # 6400 rows in chunk 2 (50 blocks)
NPHYS = NCORES * NPAD        # 100352 physical table rows
NBANK = 4
BR = NPHYS // NBANK          # 25088 rows per bank (int16-indexable)
G = 8                        # dst blocks per gather group
NG = (NB + G - 1) // G       # 13
KT = [(k, min(128, IN_DIM - k)) for k in range(0, IN_DIM, 128)]


def default_cfg():
    return dict(N=N, NCORES=NCORES, IN_DIM=IN_DIM, HID=HID, MID=MID, OUT=OUT)


def derive(cfg):
    cfg = dict(cfg)
    cfg["NPC"] = NPC
    cfg["NB"] = NB
    return cfg


def _group_blocks(g):
    return range(g * G, min((g + 1) * G, NB))


def _phys_of(node):
    c = node // NPC
    r = node % NPC
    return np.where(r < R1, c * R1 + r, NCORES * R1 + c * R2 + (r - R1))


def make_plan(edge_index):
    """Host-side graph preprocessing."""
    src = np.asarray(edge_index[0], dtype=np.int64)
    dst = np.asarray(edge_index[1], dtype=np.int64)
    # self-loops are applied on-chip via an identity matmul per block;
    # only real edges go through the gather. Degrees still count A + I.
    deg = (np.bincount(dst, minlength=N) + 1).astype(np.float64)
    dinv = 1.0 / np.sqrt(deg)

    order = np.argsort(dst, kind="stable")
    ss, dd = src[order], dst[order]
    bounds = np.searchsorted(dd, np.arange(NCORES + 1) * NPC)

    phys = _phys_of(ss)
    bank_a = phys // BR
    inb_a = (phys % BR).astype(np.int64)

    percore = []
    cnts = np.zeros((NCORES, NB * NBANK), dtype=np.int64)
    for c in range(NCORES):
        lo, hi = bounds[c], bounds[c + 1]
        dloc = dd[lo:hi] - c * NPC
        blk = dloc >> 7
        key = blk * NBANK + bank_a[lo:hi]
        cnts[c] = np.bincount(key, minlength=NB * NBANK)
        percore.append((lo, hi, dloc, key))

    nch_bq = np.ceil(cnts.max(axis=0) / 128).astype(np.int64)
    nch_bq_2d = nch_bq.reshape(NB, NBANK)
    n_b = nch_bq_2d.sum(axis=1)
    chunk_base_b = np.zeros(NB + 1, dtype=np.int64)
    chunk_base_b[1:] = np.cumsum(n_b)
    TCH = int(chunk_base_b[-1])

    S_gq = np.zeros((NG, NBANK), dtype=np.int64)
    seg_choff = np.zeros((NB, NBANK), dtype=np.int64)
    for g in range(NG):
        for q in range(NBANK):
            off = 0
            for b in _group_blocks(g):
                seg_choff[b, q] = off
                off += nch_bq_2d[b, q]
            S_gq[g, q] = off * 128
    flat = S_gq.reshape(-1)
    starts = np.zeros(NG * NBANK, dtype=np.int64)
    starts[1:] = np.cumsum(flat)[:-1]
    TOTSLOT = int(flat.sum())

    qoff_bq = np.zeros((NB, NBANK), dtype=np.int64)
    qoff_bq[:, 1:] = np.cumsum(nch_bq_2d, axis=1)[:, :-1]
    gof = np.zeros((NB, NBANK), dtype=np.int64)
    for b in range(NB):
        g = b // G
        for q in range(NBANK):
            gof[b, q] = starts[g * NBANK + q] + seg_choff[b, q] * 128

    segid = np.zeros(TOTSLOT, dtype=np.int64)
    for b in range(NB):
        for q in range(NBANK):
            s0 = gof[b, q]
            segid[s0:s0 + nch_bq_2d[b, q] * 128] = b * NBANK + q

    idx16 = np.zeros((NCORES, 128, TOTSLOT // 16), dtype=np.int16)
    bf = mybir.dt.np(BF16)
    dstL = np.full((NCORES, 128, TCH), -1.0, dtype=np.float32)
    for c in range(NCORES):
        lo, hi, dloc, key = percore[c]
        inb = inb_a[lo:hi]
        # sort by (segment, src row): the gather walks each bank region
        # in ascending order
        order2 = np.lexsort((inb, key))
        kk = key[order2]
        ii = inb[order2].astype(np.int16)
        seg_starts = np.zeros(NB * NBANK, dtype=np.int64)
        seg_starts[1:] = np.cumsum(cnts[c])[:-1]
        pos = np.arange(hi - lo) - seg_starts[kk]
        b2, q2 = kk // NBANK, kk % NBANK
        j = pos >> 7
        p = pos & 127
        t = chunk_base_b[b2] + qoff_bq[b2, q2] + j
        dstL[c, p, t] = (dloc[order2] & 127).astype(np.float32)
        s_glob = gof[b2, q2] + pos

        slotv = np.zeros(TOTSLOT, dtype=np.int16)
        slotv[s_glob] = ii
        lastv = np.zeros(NB * NBANK, dtype=np.int16)
        lastv[kk] = ii
        padmask = np.ones(TOTSLOT, dtype=bool)
        padmask[s_glob] = False
        slotv[padmask] = lastv[segid[padmask]]

        sv = slotv.reshape(TOTSLOT // 16, 16).T
        for r in range(8):
            idx16[c, 16 * r:16 * r + 16, :] = sv

    dinv2C = np.zeros((NCORES, 128, NB), dtype=np.float32)
    dinvBT = np.zeros((NCORES, 128, NPAD), dtype=bf)
    rdinvR = np.zeros((NCORES, 1, NPAD), dtype=bf)
    for c in range(NCORES):
        dv = np.zeros(NPAD, dtype=np.float32)
        dv[:NPC] = dinv[c * NPC:(c + 1) * NPC]
        dinv2C[c] = (dv * dv).reshape(NB, 128).T
        dinvBT[c] = np.broadcast_to(dv.astype(bf), (128, NPAD))
        rv = np.zeros(NPAD, dtype=np.float32)
        rv[:NPC] = np.sqrt(deg[c * NPC:(c + 1) * NPC])
        rdinvR[c] = rv.astype(bf)[None, :]

    NBMAX = int(n_b.max())
    iotaB = np.broadcast_to(
        np.arange(128, dtype=np.float32), (128, NBMAX, 128)).astype(bf)
    ident = np.eye(128, dtype=np.float32).astype(bf)
    return dict(ident=ident, TCH=TCH, TOTSLOT=TOTSLOT, NBMAX=NBMAX,
                nch_bq=nch_bq_2d, n_b=n_b, chunk_base_b=chunk_base_b,
                qoff_bq=qoff_bq, seg_choff=seg_choff, S_gq=S_gq,
                starts=starts, idx16=idx16, dstL=dstL.astype(bf),
                iotaB=iotaB, dinv2C=dinv2C, dinvBT=dinvBT, rdinvR=rdinvR)


def build_gcn(plan):
    TCH, TOTSLOT, NBMAX = plan["TCH"], plan["TOTSLOT"], plan["NBMAX"]
    nch_bq = plan["nch_bq"]
    n_b = plan["n_b"]
    chunk_base_b = plan["chunk_base_b"]
    qoff_bq = plan["qoff_bq"]
    seg_choff = plan["seg_choff"]
    S_gq = plan["S_gq"]
    starts = plan["starts"]

    nc = bacc.Bacc(None, num_swdge_queues=NBANK,
                   dynamic_dma_scratch_size=32768)
    xT = nc.dram_tensor("xT", [IN_DIM, NPAD], BF16, kind="ExternalInput")
    idx16 = nc.dram_tensor("idx16", [128, TOTSLOT // 16], I16,
                           kind="ExternalInput")
    dstL = nc.dram_tensor("dstL", [128, TCH], BF16, kind="ExternalInput")
    iotaB = nc.dram_tensor("iotaB", [128, NBMAX, 128], BF16,
                           kind="ExternalInput")
    ident = nc.dram_tensor("ident", [128, 128], BF16, kind="ExternalInput")
    w1 = nc.dram_tensor("w1", [IN_DIM, HID], BF16, kind="ExternalInput")
    w2 = nc.dram_tensor("w2", [HID, HID], BF16, kind="ExternalInput")
    wm1 = nc.dram_tensor("wm1", [HID, MID], BF16, kind="ExternalInput")
    wm2 = nc.dram_tensor("wm2", [MID, OUT], BF16, kind="ExternalInput")
    b1r = nc.dram_tensor("b1r", [1, HID], BF16, kind="ExternalInput")
    b2c = nc.dram_tensor("b2c", [HID, 1], F32, kind="ExternalInput")
    bm1c = nc.dram_tensor("bm1c", [MID, 1], F32, kind="ExternalInput")
    bm2c = nc.dram_tensor("bm2c", [OUT, 1], F32, kind="ExternalInput")
    dinv2C = nc.dram_tensor("dinv2C", [128, NB], F32, kind="ExternalInput")
    dinvBT = nc.dram_tensor("dinvBT", [128, NPAD], BF16,
                            kind="ExternalInput")
    rdinvR = nc.dram_tensor("rdinvR", [1, NPAD], BF16, kind="ExternalInput")
    outT = nc.dram_tensor("outT", [OUT, NPAD], F32, kind="ExternalOutput")

    m0_loc = nc.dram_tensor("m0_loc", [NPAD, HID], BF16)
    m0_full = nc.dram_tensor("m0_full", [NPHYS, HID], BF16,
                             addr_space="Shared")
    m1_loc = nc.dram_tensor("m1_loc", [NPAD, HID], BF16)
    m1_full = nc.dram_tensor("m1_full", [NPHYS, HID], BF16,
                             addr_space="Shared")
    cc_sem = nc.alloc_semaphore(name="cc_sem")

    # -------- phase A: table rows T1 = dinv * (x @ W1) (node slice) -----
    def phase_a():
        r0 = 0
        width = NPAD
        with tile.TileContext(nc) as tc:
            with (
                tc.tile_pool(name="w1p", bufs=1) as w1p,
                tc.tile_pool(name="xtp", bufs=1) as xtp,
                tc.tile_pool(name="hmp", bufs=2) as hmp,
                tc.tile_pool(name="h0p", bufs=2) as h0p,
                tc.tile_pool(name="psA", bufs=2, space="PSUM") as psA,
                tc.tile_pool(name="psAT", bufs=3, space="PSUM") as psAT,
            ):
                w1_sb = []
                for k0, ksz in KT:
                    t = w1p.tile([ksz, HID], BF16, tag=f"w1_{k0}")
                    nc.sync.dma_start(out=t[:], in_=w1[k0:k0 + ksz, :])
                    w1_sb.append(t)
                dinv_sb = w1p.tile([128, width], BF16, tag="dinvA")
                nc.sync.dma_start(out=dinv_sb[:],
                                  in_=dinvBT[:, r0:r0 + width])
                identA_sb = w1p.tile([128, 128], BF16, tag="identA")
                nc.sync.dma_start(out=identA_sb[:], in_=ident[:])
                xts = []
                for ti, (k0, ksz) in enumerate(KT):
                    t = xtp.tile([ksz, width], BF16, tag=f"xt_{ti}")
                    nc.sync.dma_start(out=t[:], in_=xT[k0:k0 + ksz,
                                                      r0:r0 + width])
                    xts.append(t)
                for n0 in range(0, width, 512):
                    W = min(512, width - n0)
                    nj = W // 128
                    ps = psA.tile([128, 512], F32, space="PSUM")
                    for ti in range(len(KT)):
                        nc.tensor.matmul(
                            out=ps[:, :W],
                            lhsT=w1_sb[ti][:],
                            rhs=xts[ti][:, n0:n0 + W],
                            start=(ti == 0), stop=(ti == len(KT) - 1),
                        )
                    hm = hmp.tile([128, 512], BF16, tag="hm")
                    nc.vector.tensor_tensor(
                        out=hm[:, :W], in0=ps[:, :W],
                        in1=dinv_sb[:, n0:n0 + W],
                        op=mybir.AluOpType.mult)
                    slab = h0p.tile([128, 4, HID], BF16, tag="h0slab")
                    for j in range(nj):
                        pst = psAT.tile([128, 128], BF16, space="PSUM",
                                        tag="pst", name="pst")
                        nc.tensor.transpose(
                            out=pst[:, :],
                            in_=hm[:, j * 128:(j + 1) * 128],
                            identity=identA_sb[:])
                        nc.scalar.activation(
                            out=slab[:, j, :], in_=pst[:],
                            func=mybir.ActivationFunctionType.Copy)
                    nc.sync.dma_start(
                        out=m0_loc.ap()[r0 + n0:r0 + n0 + W, :].rearrange(
                            "(j p) f -> p j f", j=nj),
                        in_=slab[:, :nj, :])
                    if r0 + n0 + W == R1:
                        # rows [0, R1) stored: ship chunk 1 while the
                        # second half of phase A computes
                        nc.gpsimd.collective_compute(
                            "AllGather", mybir.AluOpType.bypass,
                            replica_groups=[list(range(NCORES))],
                            ins=[m0_loc.ap()[0:R1, :].opt()],
                            outs=[m0_full.ap()[0:NCORES * R1, :].opt()],
                        )

    phase_a()
    nc.gpsimd.collective_compute(
        "AllGather", mybir.AluOpType.bypass,
        replica_groups=[list(range(NCORES))],
        ins=[m0_loc.ap()[R1:NPAD, :].opt()],
        outs=[m0_full.ap()[NCORES * R1:NPHYS, :].opt()],
    ).then_inc(cc_sem, 1)   # -> 1

    def gather_instr(gqp, idx_sb, cb0, table, g, q):
        slots = int(S_gq[g, q])
        t = gqp.tile([128, slots // 128, HID], BF16, tag=f"g{q}",
                     name=f"g{q}_{g}")
        base = int(starts[g * NBANK + q])
        nc.gpsimd.dma_gather(
            out_ap=t[:, :, :],
            in_ap=table.ap()[q * BR:(q + 1) * BR, :],
            idxs_ap=idx_sb[:, (base - cb0) // 16:
                           (base + slots - cb0) // 16],
            num_idxs=slots,
            num_idxs_reg=slots,
            elem_size=HID,
            elem_step=HID,
            single_packet=False,
            queue_num=q,
        )
        return t

    # ---------------- phase B: layer-1 aggregation + @W2 ----------------
    def sweep_b(glo, ghi, sem_hi):
        cb0 = int(starts[glo * NBANK])
        cb1 = (int(starts[(ghi - 1) * NBANK + NBANK - 1])
               + int(S_gq[ghi - 1, NBANK - 1]))
        t_lo = int(chunk_base_b[glo * G])
        t_hi = int(chunk_base_b[min(ghi * G, NB)])
        with tile.TileContext(nc) as tc:
            with (
                tc.tile_pool(name="metaB", bufs=1) as meta,
                tc.tile_pool(name="gB", bufs=2) as gqp,
                tc.tile_pool(name="eB", bufs=4) as eP,
                tc.tile_pool(name="psB", bufs=6, space="PSUM") as psAcc,
                tc.tile_pool(name="psB2", bufs=2, space="PSUM") as psT,
                tc.tile_pool(name="hB", bufs=3) as hP,
                tc.tile_pool(name="wB", bufs=1) as wP,
            ):
                idx_sb = meta.tile([128, (cb1 - cb0) // 16], I16)
                nc.sync.dma_start(out=idx_sb[:],
                                  in_=idx16[:, cb0 // 16:cb1 // 16])
                dstL_sb = meta.tile([128, t_hi - t_lo], BF16)
                nc.sync.dma_start(out=dstL_sb[:], in_=dstL[:, t_lo:t_hi])
                iota_sb = meta.tile([128, NBMAX, 128], BF16)
                nc.sync.dma_start(out=iota_sb[:], in_=iotaB[:])
                ident_sb = meta.tile([128, 128], BF16)
                nc.sync.dma_start(out=ident_sb[:], in_=ident[:])
                w2_sb = wP.tile([HID, HID], BF16, tag="w2")
                nc.sync.dma_start(out=w2_sb[:], in_=w2[:])
                b1r_sb = wP.tile([1, HID], BF16, tag="b1r")
                nc.sync.dma_start(out=b1r_sb[:], in_=b1r[:])
                rdinv_sb = wP.tile([1, NPAD], BF16, tag="rdinv")
                nc.sync.dma_start(out=rdinv_sb[:], in_=rdinvR[:])
                dinv2C_sb = wP.tile([128, NB], F32, tag="dinv2C")
                nc.sync.dma_start(out=dinv2C_sb[:], in_=dinv2C[:])

                gt = {}
                slab = [None]
                for g in range(glo, ghi):
                    nblk = len(list(_group_blocks(g)))
                    mloc = hP.tile([128, G, HID], BF16, tag="mloc",
                                   name="mloc")
                    r0g = g * G * 128
                    nc.sync.dma_start(
                        out=mloc[:, :nblk, :],
                        in_=m0_loc.ap()[r0g:r0g + nblk * 128, :]
                        .rearrange("(j p) f -> p j f", j=nblk))
                    for q in range(NBANK):
                        gt[g, q] = gather_instr(gqp, idx_sb, cb0, m0_full,
                                                g, q)
                    for bi, b in enumerate(_group_blocks(g)):
                        nb = int(n_b[b])
                        t0 = int(chunk_base_b[b]) - t_lo
                        eq = eP.tile([128, NBMAX, 128], BF16, tag="eq",
                                     name="eq")
                        nc.vector.tensor_tensor(
                            out=eq[:, :nb, :], in0=iota_sb[:, :nb, :],
                            in1=dstL_sb[:, t0:t0 + nb].unsqueeze(2)
                            .broadcast_to([128, nb, 128]),
                            op=mybir.AluOpType.is_equal)
                        acc = psAcc.tile([HID, 128], F32, space="PSUM")
                        # self-loop: acc[h, d] += table[d, h]
                        nc.tensor.matmul(
                            out=acc[:], lhsT=mloc[:, bi, :],
                            rhs=ident_sb[:], start=True, stop=False)
                        for q in range(NBANK):
                            for j in range(int(nch_bq[b, q])):
                                nc.tensor.matmul(
                                    out=acc[:],
                                    lhsT=gt[g, q][:, int(seg_choff[b, q])
                                                  + j, :],
                                    rhs=eq[:, int(qoff_bq[b, q]) + j, :],
                                    start=False, stop=False,
                                )
                        # += b1 (x) rdinv closes the accumulation
                        nc.tensor.matmul(
                            out=acc[:], lhsT=b1r_sb[:],
                            rhs=rdinv_sb[:, b * 128:(b + 1) * 128],
                            start=False, stop=True)
                        v = hP.tile([HID, 128], BF16, tag="v", name="v")
                        nc.scalar.activation(
                            out=v[:], in_=acc[:],
                            func=mybir.ActivationFunctionType.Copy)
                        m1ps = psT.tile([128, HID], F32, space="PSUM")
                        nc.tensor.matmul(out=m1ps[:], lhsT=v[:],
                                         rhs=w2_sb[:],
                                         start=True, stop=True)
                        jj = b % 4
                        if jj == 0:
                            slab[0] = hP.tile([128, 4, HID], BF16,
                                              tag="m1slab", name="m1slab")
                        nc.scalar.activation(
                            out=slab[0][:, jj, :], in_=m1ps[:],
                            func=mybir.ActivationFunctionType.Copy,
                            scale=dinv2C_sb[:, b:b + 1])
                        if jj == 3 or b == NB - 1:
                            rows = (jj + 1) * 128
                            base2 = (b - jj) * 128
                            nc.sync.dma_start(
                                out=m1_loc.ap()[base2:base2 + rows, :]
                                .rearrange("(j p) f -> p j f", j=jj + 1),
                                in_=slab[0][:, :jj + 1, :])
                        if b == R1 // 128 - 1:
                            # m1 rows [0, R1) stored: ship chunk 1 while
                            # the rest of the sweep runs
                            nc.gpsimd.collective_compute(
                                "AllGather", mybir.AluOpType.bypass,
                                replica_groups=[list(range(NCORES))],
                                ins=[m1_loc.ap()[0:R1, :].opt()],
                                outs=[m1_full.ap()
                                      [0:NCORES * R1, :].opt()],
                            )

    nc.gpsimd.wait_ge(cc_sem, 1)
    sweep_b(0, NG, 1)
    nc.gpsimd.collective_compute(
        "AllGather", mybir.AluOpType.bypass,
        replica_groups=[list(range(NCORES))],
        ins=[m1_loc.ap()[R1:NPAD, :].opt()],
        outs=[m1_full.ap()[NCORES * R1:NPHYS, :].opt()],
    ).then_inc(cc_sem, 1)   # -> 2

    # ---------------- phase C: layer-2 aggregation + MLP head ------------
    nc.gpsimd.wait_ge(cc_sem, 2)
    with tile.TileContext(nc) as tc:
        with (
            tc.tile_pool(name="metaC", bufs=1) as meta,
            tc.tile_pool(name="gC", bufs=2) as gqp,
            tc.tile_pool(name="eC", bufs=4) as eP,
            tc.tile_pool(name="psC", bufs=4, space="PSUM") as psAcc,
            tc.tile_pool(name="psC2", bufs=2, space="PSUM") as psC2,
            tc.tile_pool(name="psC3", bufs=2, space="PSUM") as psC3,
            tc.tile_pool(name="hC", bufs=3) as hP,
            tc.tile_pool(name="wC", bufs=1) as wP,
        ):
            idx_sb = meta.tile([128, TOTSLOT // 16], I16)
            nc.sync.dma_start(out=idx_sb[:], in_=idx16[:])
            dstL_sb = meta.tile([128, TCH], BF16)
            nc.sync.dma_start(out=dstL_sb[:], in_=dstL[:])
            iota_sb = meta.tile([128, NBMAX, 128], BF16)
            nc.sync.dma_start(out=iota_sb[:], in_=iotaB[:])
            ident_sb = meta.tile([128, 128], BF16)
            nc.sync.dma_start(out=ident_sb[:], in_=ident[:])
            wm1_sb = wP.tile([HID, MID], BF16, tag="wm1")
            nc.sync.dma_start(out=wm1_sb[:], in_=wm1[:])
            wm2_sb = wP.tile([MID, OUT], BF16, tag="wm2")
            nc.sync.dma_start(out=wm2_sb[:], in_=wm2[:])
            b2_sb = wP.tile([HID, 1], F32, tag="b2")
            nc.sync.dma_start(out=b2_sb[:], in_=b2c[:])
            bm1_sb = wP.tile([MID, 1], F32, tag="bm1")
            nc.sync.dma_start(out=bm1_sb[:], in_=bm1c[:])
            bm2_sb = wP.tile([OUT, 1], F32, tag="bm2")
            nc.sync.dma_start(out=bm2_sb[:], in_=bm2c[:])
            dinv_sb = wP.tile([128, NPAD], BF16, tag="dinvBT")
            nc.sync.dma_start(out=dinv_sb[:], in_=dinvBT[:])

            gt = {}
            zslab = [None]
            for g in range(NG):
                nblk = len(list(_group_blocks(g)))
                mloc = hP.tile([128, G, HID], BF16, tag="mloc",
                               name="mloc")
                r0g = g * G * 128
                nc.sync.dma_start(
                    out=mloc[:, :nblk, :],
                    in_=m1_loc.ap()[r0g:r0g + nblk * 128, :].rearrange(
                        "(j p) f -> p j f", j=nblk))
                for q in range(NBANK):
                    gt[g, q] = gather_instr(gqp, idx_sb, 0, m1_full, g, q)
                zslab[0] = hP.tile([OUT, G, 128], F32, tag="zslab",
                                   name="zslab")
                for bi, b in enumerate(_group_blocks(g)):
                    nb = int(n_b[b])
                    t0 = int(chunk_base_b[b])
                    eq = eP.tile([128, NBMAX, 128], BF16, tag="eq",
                                 name="eq")
                    nc.vector.tensor_tensor(
                        out=eq[:, :nb, :], in0=iota_sb[:, :nb, :],
                        in1=dstL_sb[:, t0:t0 + nb].unsqueeze(2)
                        .broadcast_to([128, nb, 128]),
                        op=mybir.AluOpType.is_equal)
                    acc = psAcc.tile([HID, 128], F32, space="PSUM")
                    # self-loop: acc[h, d] += table[d, h]
                    nc.tensor.matmul(
                        out=acc[:], lhsT=mloc[:, bi, :], rhs=ident_sb[:],
                        start=True, stop=False)
                    mm = 0
                    for q in range(NBANK):
                        for j in range(int(nch_bq[b, q])):
                            mm += 1
                            nc.tensor.matmul(
                                out=acc[:],
                                lhsT=gt[g, q][:, int(seg_choff[b, q]) + j, :],
                                rhs=eq[:, int(qoff_bq[b, q]) + j, :],
                                start=False, stop=(mm == nb),
                            )
                    # w = dinv * acc ; r2T = Relu(w + b2)
                    w = hP.tile([HID, 128], BF16, tag="w", name="w")
                    nc.vector.scalar_tensor_tensor(
                        out=w[:], in0=acc[:], scalar=0.0,
                        in1=dinv_sb[:, b * 128:(b + 1) * 128],
                        op0=mybir.AluOpType.add,
                        op1=mybir.AluOpType.mult)
                    r2T = hP.tile([HID, 128], BF16, tag="r2T", name="r2T")
                    nc.scalar.activation(
                        out=r2T[:], in_=w[:],
                        func=mybir.ActivationFunctionType.Relu,
                        bias=b2_sb[:])
                    y1 = psC2.tile([MID, 128], F32, space="PSUM")
                    nc.tensor.matmul(out=y1[:], lhsT=wm1_sb[:], rhs=r2T[:],
                                     start=True, stop=True)
                    r1T = hP.tile([MID, 128], BF16, tag="r1T", name="r1T")
                    nc.scalar.activation(
                        out=r1T[:], in_=y1[:],
                        func=mybir.ActivationFunctionType.Relu,
                        bias=bm1_sb[:])
                    z = psC3.tile([OUT, 128], F32, space="PSUM")
                    nc.tensor.matmul(out=z[:], lhsT=wm2_sb[:], rhs=r1T[:],
                                     start=True, stop=True)
                    nc.vector.tensor_scalar(
                        out=zslab[0][:, bi, :], in0=z[:], scalar1=bm2_sb[:],
                        scalar2=None, op0=mybir.AluOpType.add)
                c0 = g * G * 128
                cols = nblk * 128
                nc.sync.dma_start(
                    out=outT.ap()[:, c0:c0 + cols].rearrange(
                        "o (j p) -> o j p", j=nblk),
                    in_=zslab[0][:, :nblk, :])

    nc.compile()
    return nc


def make_inmaps(plan, inputs):
    bf = mybir.dt.np(BF16)
    x = np.asarray(inputs["x"], dtype=np.float32)
    w1 = np.asarray(inputs["w1"], np.float32).astype(bf)
    w2 = np.asarray(inputs["w2"], np.float32).astype(bf)
    wm1 = np.asarray(inputs["wm1"], np.float32).astype(bf)
    wm2 = np.asarray(inputs["wm2"], np.float32).astype(bf)
    b1r = np.asarray(inputs["b1"], np.float32).astype(bf)[None, :]
    b2c = np.asarray(inputs["b2"], np.float32)[:, None]
    bm1c = np.asarray(inputs["bm1"], np.float32)[:, None]
    bm2c = np.asarray(inputs["bm2"], np.float32)[:, None]
    in_maps = []
    for c in range(NCORES):
        xTc = np.zeros((IN_DIM, NPAD), dtype=bf)
        xTc[:, :NPC] = x[c * NPC:(c + 1) * NPC].T.astype(bf)
        in_maps.append({
            "xT": xTc, "idx16": plan["idx16"][c],
            "dstL": plan["dstL"][c], "iotaB": plan["iotaB"],
            "ident": plan["ident"],
            "w1": w1, "w2": w2, "wm1": wm1, "wm2": wm2,
            "b1r": b1r, "b2c": b2c, "bm1c": bm1c, "bm2c": bm2c,
            "dinv2C": plan["dinv2C"][c],
            "dinvBT": plan["dinvBT"][c], "rdinvR": plan["rdinvR"][c],
        })
    return in_maps


def build(inputs):
    plan = make_plan(np.asarray(inputs["edge_index"]))
    nc = build_gcn(plan)
    in_maps = make_inmaps(plan, inputs)
    return nc, in_maps


def assemble(results, cfg=None):
    return np.concatenate(
        [np.asarray(results[c]["outT"], dtype=np.float32).T[:NPC]
         for c in range(NCORES)], axis=0)


def kernel(**inputs):
    """Full-input entry point: returns [N, 4] float32."""
    nc, in_maps = build(inputs)
    from concourse.bass_utils import run_bass_kernel_spmd
    res = run_bass_kernel_spmd(nc, in_maps, core_ids=list(range(NCORES)))
    return assemble(res.results)


# revision 15
# speedup vs baseline: 1.2206x; 1.1477x over previous
"""Trainium2 Bass kernel for a 2-layer GCN (GCNConv x2 + MLP head),
8-core SPMD via run_bass_kernel_spmd.

v4 design notes (what profiling showed):
- The aggregation gather is bound by one-packet-in-flight per
  (SDMA engine, SWDGE queue) x HBM latency ~= 0.38 rows/ns with 4
  queues; the sweeps already run at that floor, so v4 hides everything
  else behind the gather stream:
  * AllGather chunks are bank-aligned (49-block chunk = banks 0-1), so
    each sweep's bank-0/1 gathers are gated only on chunk 1 and drain
    while chunk 2 is still on the wire.
  * Phase A makes W1 the stationary matmul operand (5 LDWEIGHTS per 512
    nodes instead of 5 per 128) with a [128, 512] PSUM tile, scales by
    dinv on the DVE, transposes 128x128 blocks on the DVE, and stores
    4-block slabs.
- Self-loops are applied by one identity matmul per dst block from the
  local table slice (keeps the div-banked gather segments balanced).
- Gather segments are div-banked, slots sorted by source row, pad slots
  duplicate the segment's last real row (page-hit re-reads).
"""

import numpy as np

import concourse.bacc as bacc
import concourse.mybir as mybir
import concourse.tile as tile

F32 = mybir.dt.float32
BF16 = mybir.dt.bfloat16
I16 = mybir.dt.int16

N, IN_DIM, HID, MID, OUT = 100000, 518, 128, 64, 4
NCORES = 8
NPC = N // NCORES            # 12500
NB = (NPC + 127) // 128      # 98
NPAD = NB * 128              # 12544
R1 = 48 * 128                # 6144 rows in AllGather chunk 1 (48 blocks)
R2 = NPAD - R1               # 6400 rows in chunk 2 (50 blocks)
NPHYS = NCORES * NPAD        # 100352 physical table rows
NBANK = 4
BR = NPHYS // NBANK          # 25088 rows per bank (int16-indexable)
G = 4                        # dst blocks per gather group
NG = (NB + G - 1) // G       # 25
KT = [(k, min(128, IN_DIM - k)) for k in range(0, IN_DIM, 128)]


def default_cfg():
    return dict(N=N, NCORES=NCORES, IN_DIM=IN_DIM, HID=HID, MID=MID, OUT=OUT)


def derive(cfg):
    cfg = dict(cfg)
    cfg["NPC"] = NPC
    cfg["NB"] = NB
    return cfg


def _group_blocks(g):
    return range(g * G, min((g + 1) * G, NB))


def _phys_of(node):
    c = node // NPC
    r = node % NPC
    return np.where(r < R1, c * R1 + r, NCORES * R1 + c * R2 + (r - R1))


def make_plan(edge_index):
    """Host-side graph preprocessing."""
    src = np.asarray(edge_index[0], dtype=np.int64)
    dst = np.asarray(edge_index[1], dtype=np.int64)
    # self-loops are applied on-chip via an identity matmul per block;
    # only real edges go through the gather. Degrees still count A + I.
    deg = (np.bincount(dst, minlength=N) + 1).astype(np.float64)
    dinv = 1.0 / np.sqrt(deg)

    order = np.argsort(dst, kind="stable")
    ss, dd = src[order], dst[order]
    bounds = np.searchsorted(dd, np.arange(NCORES + 1) * NPC)

    phys = _phys_of(ss)
    bank_a = phys // BR
    inb_a = (phys % BR).astype(np.int64)

    percore = []
    cnts = np.zeros((NCORES, NB * NBANK), dtype=np.int64)
    for c in range(NCORES):
        lo, hi = bounds[c], bounds[c + 1]
        dloc = dd[lo:hi] - c * NPC
        blk = dloc >> 7
        key = blk * NBANK + bank_a[lo:hi]
        cnts[c] = np.bincount(key, minlength=NB * NBANK)
        percore.append((lo, hi, dloc, key))

    nch_bq = np.ceil(cnts.max(axis=0) / 128).astype(np.int64)
    nch_bq_2d = nch_bq.reshape(NB, NBANK)
    n_b = nch_bq_2d.sum(axis=1)
    chunk_base_b = np.zeros(NB + 1, dtype=np.int64)
    chunk_base_b[1:] = np.cumsum(n_b)
    TCH = int(chunk_base_b[-1])

    S_gq = np.zeros((NG, NBANK), dtype=np.int64)
    seg_choff = np.zeros((NB, NBANK), dtype=np.int64)
    for g in range(NG):
        for q in range(NBANK):
            off = 0
            for b in _group_blocks(g):
                seg_choff[b, q] = off
                off += nch_bq_2d[b, q]
            S_gq[g, q] = off * 128
    flat = S_gq.reshape(-1)
    starts = np.zeros(NG * NBANK, dtype=np.int64)
    starts[1:] = np.cumsum(flat)[:-1]
    TOTSLOT = int(flat.sum())

    qoff_bq = np.zeros((NB, NBANK), dtype=np.int64)
    qoff_bq[:, 1:] = np.cumsum(nch_bq_2d, axis=1)[:, :-1]
    gof = np.zeros((NB, NBANK), dtype=np.int64)
    for b in range(NB):
        g = b // G
        for q in range(NBANK):
            gof[b, q] = starts[g * NBANK + q] + seg_choff[b, q] * 128

    segid = np.zeros(TOTSLOT, dtype=np.int64)
    for b in range(NB):
        for q in range(NBANK):
            s0 = gof[b, q]
            segid[s0:s0 + nch_bq_2d[b, q] * 128] = b * NBANK + q

    idx16 = np.zeros((NCORES, 128, TOTSLOT // 16), dtype=np.int16)
    bf = mybir.dt.np(BF16)
    dstL = np.full((NCORES, 128, TCH), -1.0, dtype=np.float32)
    for c in range(NCORES):
        lo, hi, dloc, key = percore[c]
        inb = inb_a[lo:hi]
        # sort by (segment, src row): the gather walks each bank region
        # in ascending order
        order2 = np.lexsort((inb, key))
        kk = key[order2]
        ii = inb[order2].astype(np.int16)
        seg_starts = np.zeros(NB * NBANK, dtype=np.int64)
        seg_starts[1:] = np.cumsum(cnts[c])[:-1]
        pos = np.arange(hi - lo) - seg_starts[kk]
        b2, q2 = kk // NBANK, kk % NBANK
        j = pos >> 7
        p = pos & 127
        t = chunk_base_b[b2] + qoff_bq[b2, q2] + j
        dstL[c, p, t] = (dloc[order2] & 127).astype(np.float32)
        s_glob = gof[b2, q2] + pos

        slotv = np.zeros(TOTSLOT, dtype=np.int16)
        slotv[s_glob] = ii
        lastv = np.zeros(NB * NBANK, dtype=np.int16)
        lastv[kk] = ii
        padmask = np.ones(TOTSLOT, dtype=bool)
        padmask[s_glob] = False
        slotv[padmask] = lastv[segid[padmask]]

        sv = slotv.reshape(TOTSLOT // 16, 16).T
        for r in range(8):
            idx16[c, 16 * r:16 * r + 16, :] = sv

    dinv2C = np.zeros((NCORES, 128, NB), dtype=np.float32)
    dinvBT = np.zeros((NCORES, 128, NPAD), dtype=bf)
    rdinvR = np.zeros((NCORES, 1, NPAD), dtype=bf)
    for c in range(NCORES):
        dv = np.zeros(NPAD, dtype=np.float32)
        dv[:NPC] = dinv[c * NPC:(c + 1) * NPC]
        dinv2C[c] = (dv * dv).reshape(NB, 128).T
        dinvBT[c] = np.broadcast_to(dv.astype(bf), (128, NPAD))
        rv = np.zeros(NPAD, dtype=np.float32)
        rv[:NPC] = np.sqrt(deg[c * NPC:(c + 1) * NPC])
        rdinvR[c] = rv.astype(bf)[None, :]

    NBMAX = int(n_b.max())
    iotaB = np.broadcast_to(
        np.arange(128, dtype=np.float32), (128, NBMAX, 128)).astype(bf)
    ident = np.eye(128, dtype=np.float32).astype(bf)
    return dict(ident=ident, TCH=TCH, TOTSLOT=TOTSLOT, NBMAX=NBMAX,
                nch_bq=nch_bq_2d, n_b=n_b, chunk_base_b=chunk_base_b,
                qoff_bq=qoff_bq, seg_choff=seg_choff, S_gq=S_gq,
                starts=starts, idx16=idx16, dstL=dstL.astype(bf),
                iotaB=iotaB, dinv2C=dinv2C, dinvBT=dinvBT, rdinvR=rdinvR)


def build_gcn(plan):
    TCH, TOTSLOT, NBMAX = plan["TCH"], plan["TOTSLOT"], plan["NBMAX"]
    nch_bq = plan["nch_bq"]
    n_b = plan["n_b"]
    chunk_base_b = plan["chunk_base_b"]
    qoff_bq = plan["qoff_bq"]
    seg_choff = plan["seg_choff"]
    S_gq = plan["S_gq"]
    starts = plan["starts"]

    nc = bacc.Bacc(None, num_swdge_queues=NBANK,
                   dynamic_dma_scratch_size=32768)
    xT = nc.dram_tensor("xT", [IN_DIM, NPAD], BF16, kind="ExternalInput")
    idx16 = nc.dram_tensor("idx16", [128, TOTSLOT // 16], I16,
                           kind="ExternalInput")
    dstL = nc.dram_tensor("dstL", [128, TCH], BF16, kind="ExternalInput")
    iotaB = nc.dram_tensor("iotaB", [128, NBMAX, 128], BF16,
                           kind="ExternalInput")
    ident = nc.dram_tensor("ident", [128, 128], BF16, kind="ExternalInput")
    w1 = nc.dram_tensor("w1", [IN_DIM, HID], BF16, kind="ExternalInput")
    w2 = nc.dram_tensor("w2", [HID, HID], BF16, kind="ExternalInput")
    wm1 = nc.dram_tensor("wm1", [HID, MID], BF16, kind="ExternalInput")
    wm2 = nc.dram_tensor("wm2", [MID, OUT], BF16, kind="ExternalInput")
    b1r = nc.dram_tensor("b1r", [1, HID], BF16, kind="ExternalInput")
    b2c = nc.dram_tensor("b2c", [HID, 1], F32, kind="ExternalInput")
    bm1c = nc.dram_tensor("bm1c", [MID, 1], F32, kind="ExternalInput")
    bm2c = nc.dram_tensor("bm2c", [OUT, 1], F32, kind="ExternalInput")
    dinv2C = nc.dram_tensor("dinv2C", [128, NB], F32, kind="ExternalInput")
    dinvBT = nc.dram_tensor("dinvBT", [128, NPAD], BF16,
                            kind="ExternalInput")
    rdinvR = nc.dram_tensor("rdinvR", [1, NPAD], BF16, kind="ExternalInput")
    outT = nc.dram_tensor("outT", [OUT, NPAD], F32, kind="ExternalOutput")

    m0_loc = nc.dram_tensor("m0_loc", [NPAD, HID], BF16)
    m0_full = nc.dram_tensor("m0_full", [NPHYS, HID], BF16,
                             addr_space="Shared")
    m1_loc = nc.dram_tensor("m1_loc", [NPAD, HID], BF16)
    m1_full = nc.dram_tensor("m1_full", [NPHYS, HID], BF16,
                             addr_space="Shared")
    cc_sem = nc.alloc_semaphore(name="cc_sem")

    # -------- phase A: table rows T1 = dinv * (x @ W1) (node slice) -----
    def phase_a():
        r0 = 0
        width = NPAD
        with tile.TileContext(nc) as tc:
            with (
                tc.tile_pool(name="w1p", bufs=1) as w1p,
                tc.tile_pool(name="xtp", bufs=1) as xtp,
                tc.tile_pool(name="hmp", bufs=2) as hmp,
                tc.tile_pool(name="h0p", bufs=2) as h0p,
                tc.tile_pool(name="psA", bufs=2, space="PSUM") as psA,
                tc.tile_pool(name="psAT", bufs=3, space="PSUM") as psAT,
            ):
                w1_sb = []
                for k0, ksz in KT:
                    t = w1p.tile([ksz, HID], BF16, tag=f"w1_{k0}")
                    nc.sync.dma_start(out=t[:], in_=w1[k0:k0 + ksz, :])
                    w1_sb.append(t)
                dinv_sb = w1p.tile([128, width], BF16, tag="dinvA")
                nc.sync.dma_start(out=dinv_sb[:],
                                  in_=dinvBT[:, r0:r0 + width])
                identA_sb = w1p.tile([128, 128], BF16, tag="identA")
                nc.sync.dma_start(out=identA_sb[:], in_=ident[:])
                xts = []
                for ti, (k0, ksz) in enumerate(KT):
                    t = xtp.tile([ksz, width], BF16, tag=f"xt_{ti}")
                    nc.sync.dma_start(out=t[:], in_=xT[k0:k0 + ksz,
                                                      r0:r0 + width])
                    xts.append(t)
                for n0 in range(0, width, 512):
                    W = min(512, width - n0)
                    nj = W // 128
                    ps = psA.tile([128, 512], F32, space="PSUM")
                    for ti in range(len(KT)):
                        nc.tensor.matmul(
                            out=ps[:, :W],
                            lhsT=w1_sb[ti][:],
                            rhs=xts[ti][:, n0:n0 + W],
                            start=(ti == 0), stop=(ti == len(KT) - 1),
                        )
                    hm = hmp.tile([128, 512], BF16, tag="hm")
                    nc.vector.tensor_tensor(
                        out=hm[:, :W], in0=ps[:, :W],
                        in1=dinv_sb[:, n0:n0 + W],
                        op=mybir.AluOpType.mult)
                    slab = h0p.tile([128, 4, HID], BF16, tag="h0slab")
                    for j in range(nj):
                        pst = psAT.tile([128, 128], BF16, space="PSUM",
                                        tag="pst", name="pst")
                        nc.tensor.transpose(
                            out=pst[:, :],
                            in_=hm[:, j * 128:(j + 1) * 128],
                            identity=identA_sb[:])
                        nc.scalar.activation(
                            out=slab[:, j, :], in_=pst[:],
                            func=mybir.ActivationFunctionType.Copy)
                    nc.sync.dma_start(
                        out=m0_loc.ap()[r0 + n0:r0 + n0 + W, :].rearrange(
                            "(j p) f -> p j f", j=nj),
                        in_=slab[:, :nj, :])
                    if r0 + n0 + W == R1:
                        # rows [0, R1) stored: ship chunk 1 while the
                        # second half of phase A computes
                        nc.gpsimd.collective_compute(
                            "AllGather", mybir.AluOpType.bypass,
                            replica_groups=[list(range(NCORES))],
                            ins=[m0_loc.ap()[0:R1, :].opt()],
                            outs=[m0_full.ap()[0:NCORES * R1, :].opt()],
                        )

    phase_a()
    nc.gpsimd.collective_compute(
        "AllGather", mybir.AluOpType.bypass,
        replica_groups=[list(range(NCORES))],
        ins=[m0_loc.ap()[R1:NPAD, :].opt()],
        outs=[m0_full.ap()[NCORES * R1:NPHYS, :].opt()],
    ).then_inc(cc_sem, 1)   # -> 1

    def gather_instr(gqp, idx_sb, table, g, q):
        slots = int(S_gq[g, q])
        t = gqp.tile([128, slots // 128, HID], BF16, tag=f"g{q}",
                     name=f"g{q}_{g}")
        base = int(starts[g * NBANK + q])
        nc.gpsimd.dma_gather(
            out_ap=t[:, :, :],
            in_ap=table.ap()[q * BR:(q + 1) * BR, :],
            idxs_ap=idx_sb[:, base // 16:(base + slots) // 16],
            num_idxs=slots,
            num_idxs_reg=slots,
            elem_size=HID,
            elem_step=HID,
            single_packet=False,
            queue_num=q,
        )
        return t

    # ------- phases B + C: both aggregation sweeps in one context -------
    # The m1 AllGather chunks are issued in-context; Tile's shadow-memory
    # DRAM tracking orders the layer-2 gathers after them.
    nc.gpsimd.wait_ge(cc_sem, 1)
    with tile.TileContext(nc) as tc:
        with (
            tc.tile_pool(name="meta", bufs=1) as meta,
            tc.tile_pool(name="wS", bufs=1) as wS,
            tc.tile_pool(name="wG", bufs=2) as wG,
            tc.tile_pool(name="gq", bufs=4) as gqp,
            tc.tile_pool(name="eP", bufs=3) as eP,
            tc.tile_pool(name="mlp", bufs=2) as mlp,
            tc.tile_pool(name="hS", bufs=3) as hS,
            tc.tile_pool(name="slabp", bufs=2) as slabp,
            tc.tile_pool(name="psAcc", bufs=3, space="PSUM") as psAcc,
            tc.tile_pool(name="psM", bufs=2, space="PSUM") as psM,
            tc.tile_pool(name="psY", bufs=2, space="PSUM") as psY,
            tc.tile_pool(name="psZ", bufs=1, space="PSUM") as psZ,
        ):
            idx_sb = meta.tile([128, TOTSLOT // 16], I16)
            nc.sync.dma_start(out=idx_sb[:], in_=idx16[:])
            dstL_sb = meta.tile([128, TCH], BF16)
            nc.sync.dma_start(out=dstL_sb[:], in_=dstL[:])
            iota_sb = meta.tile([128, NBMAX, 128], BF16)
            nc.sync.dma_start(out=iota_sb[:], in_=iotaB[:])
            ident_sb = meta.tile([128, 128], BF16)
            nc.sync.dma_start(out=ident_sb[:], in_=ident[:])
            w2_sb = wS.tile([HID, HID], BF16, tag="w2")
            nc.sync.dma_start(out=w2_sb[:], in_=w2[:])
            b1r_sb = wS.tile([1, HID], BF16, tag="b1r")
            nc.sync.dma_start(out=b1r_sb[:], in_=b1r[:])
            dinv2C_sb = wS.tile([128, NB], F32, tag="dinv2C")
            nc.sync.dma_start(out=dinv2C_sb[:], in_=dinv2C[:])
            wm1_sb = wS.tile([HID, MID], BF16, tag="wm1")
            nc.sync.dma_start(out=wm1_sb[:], in_=wm1[:])
            wm2_sb = wS.tile([MID, OUT], BF16, tag="wm2")
            nc.sync.dma_start(out=wm2_sb[:], in_=wm2[:])
            b2_sb = wS.tile([HID, 1], F32, tag="b2")
            nc.sync.dma_start(out=b2_sb[:], in_=b2c[:])
            bm1_sb = wS.tile([MID, 1], F32, tag="bm1")
            nc.sync.dma_start(out=bm1_sb[:], in_=bm1c[:])
            bm2_sb = wS.tile([OUT, 1], F32, tag="bm2")
            nc.sync.dma_start(out=bm2_sb[:], in_=bm2c[:])

            gt = {}

            def agg_block(g, bi, b, mloc, layer):
                nb = int(n_b[b])
                t0 = int(chunk_base_b[b])
                eq = eP.tile([128, NBMAX, 128], BF16, tag="eq", name="eq")
                nc.vector.tensor_tensor(
                    out=eq[:, :nb, :], in0=iota_sb[:, :nb, :],
                    in1=dstL_sb[:, t0:t0 + nb].unsqueeze(2)
                    .broadcast_to([128, nb, 128]),
                    op=mybir.AluOpType.is_equal)
                acc = psAcc.tile([HID, 128], F32, space="PSUM")
                # self-loop: acc[h, d] += table[d, h]
                nc.tensor.matmul(
                    out=acc[:], lhsT=mloc[:, bi, :], rhs=ident_sb[:],
                    start=True, stop=False)
                mm = 0
                last = (layer == 2)
                for q in range(NBANK):
                    for j in range(int(nch_bq[b, q])):
                        mm += 1
                        nc.tensor.matmul(
                            out=acc[:],
                            lhsT=gt[layer, g, q][:, int(seg_choff[b, q])
                                                 + j, :],
                            rhs=eq[:, int(qoff_bq[b, q]) + j, :],
                            start=False, stop=(last and mm == nb),
                        )
                return acc

            # ---------------- layer-1 sweep ----------------
            slab = [None]
            for g in range(NG):
                nblk = len(list(_group_blocks(g)))
                mloc = mlp.tile([128, G, HID], BF16, tag="mloc",
                                name="mloc")
                r0g = g * G * 128
                nc.sync.dma_start(
                    out=mloc[:, :nblk, :],
                    in_=m0_loc.ap()[r0g:r0g + nblk * 128, :]
                    .rearrange("(j p) f -> p j f", j=nblk))
                rdg = wG.tile([1, G * 128], BF16, tag="rdg", name="rdg")
                nc.sync.dma_start(out=rdg[:, :nblk * 128],
                                  in_=rdinvR[:, r0g:r0g + nblk * 128])
                for q in range(NBANK):
                    gt[1, g, q] = gather_instr(gqp, idx_sb, m0_full, g, q)
                for bi, b in enumerate(_group_blocks(g)):
                    acc = agg_block(g, bi, b, mloc, 1)
                    # += b1 (x) rdinv closes the accumulation
                    nc.tensor.matmul(
                        out=acc[:], lhsT=b1r_sb[:],
                        rhs=rdg[:, bi * 128:(bi + 1) * 128],
                        start=False, stop=True)
                    v = hS.tile([HID, 128], BF16, tag="v", name="v")
                    nc.scalar.activation(
                        out=v[:], in_=acc[:],
                        func=mybir.ActivationFunctionType.Copy)
                    m1ps = psM.tile([128, HID], F32, space="PSUM")
                    nc.tensor.matmul(out=m1ps[:], lhsT=v[:], rhs=w2_sb[:],
                                     start=True, stop=True)
                    jj = b % 4
                    if jj == 0:
                        slab[0] = hS.tile([128, 4, HID], BF16,
                                          tag="m1slab", name="m1slab")
                    nc.scalar.activation(
                        out=slab[0][:, jj, :], in_=m1ps[:],
                        func=mybir.ActivationFunctionType.Copy,
                        scale=dinv2C_sb[:, b:b + 1])
                    if jj == 3 or b == NB - 1:
                        rows = (jj + 1) * 128
                        base2 = (b - jj) * 128
                        nc.sync.dma_start(
                            out=m1_loc.ap()[base2:base2 + rows, :]
                            .rearrange("(j p) f -> p j f", j=jj + 1),
                            in_=slab[0][:, :jj + 1, :])
                    if b == R1 // 128 - 1:
                        # m1 rows [0, R1) stored: ship chunk 1 now
                        nc.gpsimd.collective_compute(
                            "AllGather", mybir.AluOpType.bypass,
                            replica_groups=[list(range(NCORES))],
                            ins=[m1_loc.ap()[0:R1, :].opt()],
                            outs=[m1_full.ap()[0:NCORES * R1, :].opt()],
                        )
                    if b == NB - 1:
                        nc.gpsimd.collective_compute(
                            "AllGather", mybir.AluOpType.bypass,
                            replica_groups=[list(range(NCORES))],
                            ins=[m1_loc.ap()[R1:NPAD, :].opt()],
                            outs=[m1_full.ap()
                                  [NCORES * R1:NPHYS, :].opt()],
                        )

            # ---------------- layer-2 sweep + MLP head ----------------
            for g in range(NG):
                nblk = len(list(_group_blocks(g)))
                mloc = mlp.tile([128, G, HID], BF16, tag="mloc",
                                name="mloc")
                r0g = g * G * 128
                nc.sync.dma_start(
                    out=mloc[:, :nblk, :],
                    in_=m1_loc.ap()[r0g:r0g + nblk * 128, :]
                    .rearrange("(j p) f -> p j f", j=nblk))
                dvg = wG.tile([128, G * 128], BF16, tag="dvg", name="dvg")
                nc.sync.dma_start(out=dvg[:, :nblk * 128],
                                  in_=dinvBT[:, r0g:r0g + nblk * 128])
                for q in range(NBANK):
                    gt[2, g, q] = gather_instr(gqp, idx_sb, m1_full, g, q)
                zslab = slabp.tile([OUT, G, 128], F32, tag="zslab",
                                   name="zslab")
                for bi, b in enumerate(_group_blocks(g)):
                    acc = agg_block(g, bi, b, mloc, 2)
                    # w = dinv * acc ; r2T = Relu(w + b2)
                    w = hS.tile([HID, 128], BF16, tag="w", name="w")
                    nc.vector.scalar_tensor_tensor(
                        out=w[:], in0=acc[:], scalar=0.0,
                        in1=dvg[:, bi * 128:(bi + 1) * 128],
                        op0=mybir.AluOpType.add,
                        op1=mybir.AluOpType.mult)
                    r2T = hS.tile([HID, 128], BF16, tag="r2T", name="r2T")
                    nc.scalar.activation(
                        out=r2T[:], in_=w[:],
                        func=mybir.ActivationFunctionType.Relu,
                        bias=b2_sb[:])
                    y1 = psY.tile([MID, 128], F32, space="PSUM")
                    nc.tensor.matmul(out=y1[:], lhsT=wm1_sb[:],
                                     rhs=r2T[:], start=True, stop=True)
                    r1T = hS.tile([MID, 128], BF16, tag="r1T", name="r1T")
                    nc.scalar.activation(
                        out=r1T[:], in_=y1[:],
                        func=mybir.ActivationFunctionType.Relu,
                        bias=bm1_sb[:])
                    z = psZ.tile([OUT, 128], F32, space="PSUM")
                    nc.tensor.matmul(out=z[:], lhsT=wm2_sb[:], rhs=r1T[:],
                                     start=True, stop=True)
                    nc.vector.tensor_scalar(
                        out=zslab[:, bi, :], in0=z[:], scalar1=bm2_sb[:],
                        scalar2=None, op0=mybir.AluOpType.add)
                c0 = g * G * 128
                cols = nblk * 128
                nc.sync.dma_start(
                    out=outT.ap()[:, c0:c0 + cols].rearrange(
                        "o (j p) -> o j p", j=nblk),
                    in_=zslab[:, :nblk, :])

    nc.compile()
    return nc


def make_inmaps(plan, inputs):
    bf = mybir.dt.np(BF16)
    x = np.asarray(inputs["x"], dtype=np.float32)
    w1 = np.asarray(inputs["w1"], np.float32).astype(bf)
    w2 = np.asarray(inputs["w2"], np.float32).astype(bf)
    wm1 = np.asarray(inputs["wm1"], np.float32).astype(bf)
    wm2 = np.asarray(inputs["wm2"], np.float32).astype(bf)
    b1r = np.asarray(inputs["b1"], np.float32).astype(bf)[None, :]
    b2c = np.asarray(inputs["b2"], np.float32)[:, None]
    bm1c = np.asarray(inputs["bm1"], np.float32)[:, None]
    bm2c = np.asarray(inputs["bm2"], np.float32)[:, None]
    in_maps = []
    for c in range(NCORES):
        xTc = np.zeros((IN_DIM, NPAD), dtype=bf)
        xTc[:, :NPC] = x[c * NPC:(c + 1) * NPC].T.astype(bf)
        in_maps.append({
            "xT": xTc, "idx16": plan["idx16"][c],
            "dstL": plan["dstL"][c], "iotaB": plan["iotaB"],
            "ident": plan["ident"],
            "w1": w1, "w2": w2, "wm1": wm1, "wm2": wm2,
            "b1r": b1r, "b2c": b2c, "bm1c": bm1c, "bm2c": bm2c,
            "dinv2C": plan["dinv2C"][c],
            "dinvBT": plan["dinvBT"][c], "rdinvR": plan["rdinvR"][c],
        })
    return in_maps


def build(inputs):
    plan = make_plan(np.asarray(inputs["edge_index"]))
    nc = build_gcn(plan)
    in_maps = make_inmaps(plan, inputs)
    return nc, in_maps


def assemble(results, cfg=None):
    return np.concatenate(
        [np.asarray(results[c]["outT"], dtype=np.float32).T[:NPC]
         for c in range(NCORES)], axis=0)


def kernel(**inputs):
    """Full-input entry point: returns [N, 4] float32."""
    nc, in_maps = build(inputs)
    from concourse.bass_utils import run_bass_kernel_spmd
    res = run_bass_kernel_spmd(nc, in_maps, core_ids=list(range(NCORES)))
    return assemble(res.results)


# revision 16
# speedup vs baseline: 1.2788x; 1.0476x over previous
"""Trainium2 Bass kernel for a 2-layer GCN (GCNConv x2 + MLP head),
8-core SPMD via run_bass_kernel_spmd.

v4 design notes (what profiling showed):
- The aggregation gather is bound by one-packet-in-flight per
  (SDMA engine, SWDGE queue) x HBM latency ~= 0.38 rows/ns with 4
  queues; the sweeps already run at that floor, so v4 hides everything
  else behind the gather stream:
  * AllGather chunks are bank-aligned (49-block chunk = banks 0-1), so
    each sweep's bank-0/1 gathers are gated only on chunk 1 and drain
    while chunk 2 is still on the wire.
  * Phase A makes W1 the stationary matmul operand (5 LDWEIGHTS per 512
    nodes instead of 5 per 128) with a [128, 512] PSUM tile, scales by
    dinv on the DVE, transposes 128x128 blocks on the DVE, and stores
    4-block slabs.
- Self-loops are applied by one identity matmul per dst block from the
  local table slice (keeps the div-banked gather segments balanced).
- Gather segments are div-banked, slots sorted by source row, pad slots
  duplicate the segment's last real row (page-hit re-reads).
"""

import numpy as np

import concourse.bacc as bacc
import concourse.mybir as mybir
import concourse.tile as tile

F32 = mybir.dt.float32
BF16 = mybir.dt.bfloat16
I16 = mybir.dt.int16

N, IN_DIM, HID, MID, OUT = 100000, 518, 128, 64, 4
NCORES = 8
NPC = N // NCORES            # 12500
NB = (NPC + 127) // 128      # 98
NPAD = NB * 128              # 12544
R1 = 49 * 128                # 6272 rows in AllGather chunk 1 (49 blocks)
R2 = NPAD - R1               # 6272 rows in chunk 2 (49 blocks)
NPHYS = NCORES * NPAD        # 100352 physical table rows
NBANK = 4
BR = NPHYS // NBANK          # 25088 rows per bank (int16-indexable)
# chunk 1 occupies exactly banks 0-1: sweeps' bank-0/1 gathers depend
# only on chunk 1 via Tile's range-based DRAM tracking
assert NCORES * R1 == 2 * BR
G = 4                        # dst blocks per gather group
NG = (NB + G - 1) // G       # 25
KT = [(k, min(128, IN_DIM - k)) for k in range(0, IN_DIM, 128)]


def default_cfg():
    return dict(N=N, NCORES=NCORES, IN_DIM=IN_DIM, HID=HID, MID=MID, OUT=OUT)


def derive(cfg):
    cfg = dict(cfg)
    cfg["NPC"] = NPC
    cfg["NB"] = NB
    return cfg


def _group_blocks(g):
    return range(g * G, min((g + 1) * G, NB))


def _phys_of(node):
    c = node // NPC
    r = node % NPC
    return np.where(r < R1, c * R1 + r, NCORES * R1 + c * R2 + (r - R1))


def make_plan(edge_index):
    """Host-side graph preprocessing."""
    src = np.asarray(edge_index[0], dtype=np.int64)
    dst = np.asarray(edge_index[1], dtype=np.int64)
    # self-loops are applied on-chip via an identity matmul per block;
    # only real edges go through the gather. Degrees still count A + I.
    deg = (np.bincount(dst, minlength=N) + 1).astype(np.float64)
    dinv = 1.0 / np.sqrt(deg)

    order = np.argsort(dst, kind="stable")
    ss, dd = src[order], dst[order]
    bounds = np.searchsorted(dd, np.arange(NCORES + 1) * NPC)

    phys = _phys_of(ss)
    bank_a = phys // BR
    inb_a = (phys % BR).astype(np.int64)

    percore = []
    cnts = np.zeros((NCORES, NB * NBANK), dtype=np.int64)
    for c in range(NCORES):
        lo, hi = bounds[c], bounds[c + 1]
        dloc = dd[lo:hi] - c * NPC
        blk = dloc >> 7
        key = blk * NBANK + bank_a[lo:hi]
        cnts[c] = np.bincount(key, minlength=NB * NBANK)
        percore.append((lo, hi, dloc, key))

    nch_bq = np.ceil(cnts.max(axis=0) / 128).astype(np.int64)
    nch_bq_2d = nch_bq.reshape(NB, NBANK)
    n_b = nch_bq_2d.sum(axis=1)
    chunk_base_b = np.zeros(NB + 1, dtype=np.int64)
    chunk_base_b[1:] = np.cumsum(n_b)
    TCH = int(chunk_base_b[-1])

    S_gq = np.zeros((NG, NBANK), dtype=np.int64)
    seg_choff = np.zeros((NB, NBANK), dtype=np.int64)
    for g in range(NG):
        for q in range(NBANK):
            off = 0
            for b in _group_blocks(g):
                seg_choff[b, q] = off
                off += nch_bq_2d[b, q]
            S_gq[g, q] = off * 128
    flat = S_gq.reshape(-1)
    starts = np.zeros(NG * NBANK, dtype=np.int64)
    starts[1:] = np.cumsum(flat)[:-1]
    TOTSLOT = int(flat.sum())

    qoff_bq = np.zeros((NB, NBANK), dtype=np.int64)
    qoff_bq[:, 1:] = np.cumsum(nch_bq_2d, axis=1)[:, :-1]
    gof = np.zeros((NB, NBANK), dtype=np.int64)
    for b in range(NB):
        g = b // G
        for q in range(NBANK):
            gof[b, q] = starts[g * NBANK + q] + seg_choff[b, q] * 128

    segid = np.zeros(TOTSLOT, dtype=np.int64)
    for b in range(NB):
        for q in range(NBANK):
            s0 = gof[b, q]
            segid[s0:s0 + nch_bq_2d[b, q] * 128] = b * NBANK + q

    idx16 = np.zeros((NCORES, 128, TOTSLOT // 16), dtype=np.int16)
    bf = mybir.dt.np(BF16)
    dstL = np.full((NCORES, 128, TCH), -1.0, dtype=np.float32)
    for c in range(NCORES):
        lo, hi, dloc, key = percore[c]
        inb = inb_a[lo:hi]
        # sort by (segment, src row): the gather walks each bank region
        # in ascending order
        order2 = np.lexsort((inb, key))
        kk = key[order2]
        ii = inb[order2].astype(np.int16)
        seg_starts = np.zeros(NB * NBANK, dtype=np.int64)
        seg_starts[1:] = np.cumsum(cnts[c])[:-1]
        pos = np.arange(hi - lo) - seg_starts[kk]
        b2, q2 = kk // NBANK, kk % NBANK
        j = pos >> 7
        p = pos & 127
        t = chunk_base_b[b2] + qoff_bq[b2, q2] + j
        dstL[c, p, t] = (dloc[order2] & 127).astype(np.float32)
        s_glob = gof[b2, q2] + pos

        slotv = np.zeros(TOTSLOT, dtype=np.int16)
        slotv[s_glob] = ii
        lastv = np.zeros(NB * NBANK, dtype=np.int16)
        lastv[kk] = ii
        padmask = np.ones(TOTSLOT, dtype=bool)
        padmask[s_glob] = False
        slotv[padmask] = lastv[segid[padmask]]

        sv = slotv.reshape(TOTSLOT // 16, 16).T
        for r in range(8):
            idx16[c, 16 * r:16 * r + 16, :] = sv

    dinv2C = np.zeros((NCORES, 128, NB), dtype=np.float32)
    dinvBT = np.zeros((NCORES, 128, NPAD), dtype=bf)
    rdinvR = np.zeros((NCORES, 1, NPAD), dtype=bf)
    for c in range(NCORES):
        dv = np.zeros(NPAD, dtype=np.float32)
        dv[:NPC] = dinv[c * NPC:(c + 1) * NPC]
        dinv2C[c] = (dv * dv).reshape(NB, 128).T
        dinvBT[c] = np.broadcast_to(dv.astype(bf), (128, NPAD))
        rv = np.zeros(NPAD, dtype=np.float32)
        rv[:NPC] = np.sqrt(deg[c * NPC:(c + 1) * NPC])
        rdinvR[c] = rv.astype(bf)[None, :]

    NBMAX = int(n_b.max())
    iotaB = np.broadcast_to(
        np.arange(128, dtype=np.float32), (128, NBMAX, 128)).astype(bf)
    ident = np.eye(128, dtype=np.float32).astype(bf)
    return dict(ident=ident, TCH=TCH, TOTSLOT=TOTSLOT, NBMAX=NBMAX,
                nch_bq=nch_bq_2d, n_b=n_b, chunk_base_b=chunk_base_b,
                qoff_bq=qoff_bq, seg_choff=seg_choff, S_gq=S_gq,
                starts=starts, idx16=idx16, dstL=dstL.astype(bf),
                iotaB=iotaB, dinv2C=dinv2C, dinvBT=dinvBT, rdinvR=rdinvR)


def build_gcn(plan):
    TCH, TOTSLOT, NBMAX = plan["TCH"], plan["TOTSLOT"], plan["NBMAX"]
    nch_bq = plan["nch_bq"]
    n_b = plan["n_b"]
    chunk_base_b = plan["chunk_base_b"]
    qoff_bq = plan["qoff_bq"]
    seg_choff = plan["seg_choff"]
    S_gq = plan["S_gq"]
    starts = plan["starts"]

    nc = bacc.Bacc(None, num_swdge_queues=NBANK,
                   dynamic_dma_scratch_size=32768)
    xT = nc.dram_tensor("xT", [IN_DIM, NPAD], BF16, kind="ExternalInput")
    idx16 = nc.dram_tensor("idx16", [128, TOTSLOT // 16], I16,
                           kind="ExternalInput")
    dstL = nc.dram_tensor("dstL", [128, TCH], BF16, kind="ExternalInput")
    iotaB = nc.dram_tensor("iotaB", [128, NBMAX, 128], BF16,
                           kind="ExternalInput")
    ident = nc.dram_tensor("ident", [128, 128], BF16, kind="ExternalInput")
    w1 = nc.dram_tensor("w1", [IN_DIM, HID], BF16, kind="ExternalInput")
    w2 = nc.dram_tensor("w2", [HID, HID], BF16, kind="ExternalInput")
    wm1 = nc.dram_tensor("wm1", [HID, MID], BF16, kind="ExternalInput")
    wm2 = nc.dram_tensor("wm2", [MID, OUT], BF16, kind="ExternalInput")
    b1r = nc.dram_tensor("b1r", [1, HID], BF16, kind="ExternalInput")
    b2c = nc.dram_tensor("b2c", [HID, 1], F32, kind="ExternalInput")
    bm1c = nc.dram_tensor("bm1c", [MID, 1], F32, kind="ExternalInput")
    bm2c = nc.dram_tensor("bm2c", [OUT, 1], F32, kind="ExternalInput")
    dinv2C = nc.dram_tensor("dinv2C", [128, NB], F32, kind="ExternalInput")
    dinvBT = nc.dram_tensor("dinvBT", [128, NPAD], BF16,
                            kind="ExternalInput")
    rdinvR = nc.dram_tensor("rdinvR", [1, NPAD], BF16, kind="ExternalInput")
    outT = nc.dram_tensor("outT", [OUT, NPAD], F32, kind="ExternalOutput")

    m0_loc = nc.dram_tensor("m0_loc", [NPAD, HID], BF16)
    m0_full = nc.dram_tensor("m0_full", [NPHYS, HID], BF16,
                             addr_space="Shared")
    m1_loc = nc.dram_tensor("m1_loc", [NPAD, HID], BF16)
    m1_full = nc.dram_tensor("m1_full", [NPHYS, HID], BF16,
                             addr_space="Shared")
    cc_sem = nc.alloc_semaphore(name="cc_sem")

    # -------- phase A: table rows T1 = dinv * (x @ W1) (node slice) -----
    def phase_a():
        r0 = 0
        width = NPAD
        with tile.TileContext(nc) as tc:
            with (
                tc.tile_pool(name="w1p", bufs=1) as w1p,
                tc.tile_pool(name="xtp", bufs=1) as xtp,
                tc.tile_pool(name="hmp", bufs=3) as hmp,
                tc.tile_pool(name="h0p", bufs=3) as h0p,
                tc.tile_pool(name="psA", bufs=3, space="PSUM") as psA,
                tc.tile_pool(name="psAT", bufs=4, space="PSUM") as psAT,
            ):
                w1_sb = []
                for k0, ksz in KT:
                    t = w1p.tile([ksz, HID], BF16, tag=f"w1_{k0}")
                    nc.sync.dma_start(out=t[:], in_=w1[k0:k0 + ksz, :])
                    w1_sb.append(t)
                dinv_sb = w1p.tile([128, width], BF16, tag="dinvA")
                nc.sync.dma_start(out=dinv_sb[:],
                                  in_=dinvBT[:, r0:r0 + width])
                identA_sb = w1p.tile([128, 128], BF16, tag="identA")
                nc.sync.dma_start(out=identA_sb[:], in_=ident[:])
                xts = []
                for ti, (k0, ksz) in enumerate(KT):
                    t = xtp.tile([ksz, width], BF16, tag=f"xt_{ti}")
                    nc.sync.dma_start(out=t[:], in_=xT[k0:k0 + ksz,
                                                      r0:r0 + width])
                    xts.append(t)
                for n0 in range(0, width, 512):
                    W = min(512, width - n0)
                    nj = W // 128
                    ps = psA.tile([128, 512], F32, space="PSUM")
                    for ti in range(len(KT)):
                        nc.tensor.matmul(
                            out=ps[:, :W],
                            lhsT=w1_sb[ti][:],
                            rhs=xts[ti][:, n0:n0 + W],
                            start=(ti == 0), stop=(ti == len(KT) - 1),
                        )
                    hm = hmp.tile([128, 512], BF16, tag="hm")
                    nc.vector.tensor_tensor(
                        out=hm[:, :W], in0=ps[:, :W],
                        in1=dinv_sb[:, n0:n0 + W],
                        op=mybir.AluOpType.mult)
                    slab = h0p.tile([128, 4, HID], BF16, tag="h0slab")
                    for j in range(nj):
                        pst = psAT.tile([128, 128], BF16, space="PSUM",
                                        tag="pst", name="pst")
                        nc.tensor.transpose(
                            out=pst[:, :],
                            in_=hm[:, j * 128:(j + 1) * 128],
                            identity=identA_sb[:])
                        nc.scalar.activation(
                            out=slab[:, j, :], in_=pst[:],
                            func=mybir.ActivationFunctionType.Copy)
                    nc.sync.dma_start(
                        out=m0_loc.ap()[r0 + n0:r0 + n0 + W, :].rearrange(
                            "(j p) f -> p j f", j=nj),
                        in_=slab[:, :nj, :])
                    if r0 + n0 < R1 <= r0 + n0 + W:
                        # rows [0, R1) stored: ship chunk 1 while the
                        # second half of phase A computes
                        nc.gpsimd.collective_compute(
                            "AllGather", mybir.AluOpType.bypass,
                            replica_groups=[list(range(NCORES))],
                            ins=[m0_loc.ap()[0:R1, :].opt()],
                            outs=[m0_full.ap()[0:NCORES * R1, :].opt()],
                        )

    phase_a()

    def gather_instr(gqp, idx_sb, table, g, q):
        slots = int(S_gq[g, q])
        t = gqp.tile([128, slots // 128, HID], BF16, tag=f"g{q}",
                     name=f"g{q}_{g}")
        base = int(starts[g * NBANK + q])
        nc.gpsimd.dma_gather(
            out_ap=t[:, :, :],
            in_ap=table.ap()[q * BR:(q + 1) * BR, :],
            idxs_ap=idx_sb[:, base // 16:(base + slots) // 16],
            num_idxs=slots,
            num_idxs_reg=slots,
            elem_size=HID,
            elem_step=HID,
            single_packet=False,
            queue_num=q,
        )
        return t

    # ------- phases B + C: both aggregation sweeps in one context -------
    # All remaining collectives are in-context; Tile's range-based DRAM
    # tracking gates each gather on exactly the table chunks it reads.
    with tile.TileContext(nc) as tc:
        with (
            tc.tile_pool(name="meta", bufs=1) as meta,
            tc.tile_pool(name="wS", bufs=1) as wS,
            tc.tile_pool(name="wG", bufs=2) as wG,
            tc.tile_pool(name="gq", bufs=4) as gqp,
            tc.tile_pool(name="eP", bufs=3) as eP,
            tc.tile_pool(name="mlp", bufs=2) as mlp,
            tc.tile_pool(name="hS", bufs=3) as hS,
            tc.tile_pool(name="slabp", bufs=2) as slabp,
            tc.tile_pool(name="psAcc", bufs=3, space="PSUM") as psAcc,
            tc.tile_pool(name="psM", bufs=2, space="PSUM") as psM,
            tc.tile_pool(name="psY", bufs=2, space="PSUM") as psY,
            tc.tile_pool(name="psZ", bufs=1, space="PSUM") as psZ,
        ):
            idx_sb = meta.tile([128, TOTSLOT // 16], I16)
            nc.sync.dma_start(out=idx_sb[:], in_=idx16[:])
            dstL_sb = meta.tile([128, TCH], BF16)
            nc.sync.dma_start(out=dstL_sb[:], in_=dstL[:])
            iota_sb = meta.tile([128, NBMAX, 128], BF16)
            nc.sync.dma_start(out=iota_sb[:], in_=iotaB[:])
            ident_sb = meta.tile([128, 128], BF16)
            nc.sync.dma_start(out=ident_sb[:], in_=ident[:])
            w2_sb = wS.tile([HID, HID], BF16, tag="w2")
            nc.sync.dma_start(out=w2_sb[:], in_=w2[:])
            b1r_sb = wS.tile([1, HID], BF16, tag="b1r")
            nc.sync.dma_start(out=b1r_sb[:], in_=b1r[:])
            dinv2C_sb = wS.tile([128, NB], F32, tag="dinv2C")
            nc.sync.dma_start(out=dinv2C_sb[:], in_=dinv2C[:])
            wm1_sb = wS.tile([HID, MID], BF16, tag="wm1")
            nc.sync.dma_start(out=wm1_sb[:], in_=wm1[:])
            wm2_sb = wS.tile([MID, OUT], BF16, tag="wm2")
            nc.sync.dma_start(out=wm2_sb[:], in_=wm2[:])
            b2_sb = wS.tile([HID, 1], F32, tag="b2")
            nc.sync.dma_start(out=b2_sb[:], in_=b2c[:])
            bm1_sb = wS.tile([MID, 1], F32, tag="bm1")
            nc.sync.dma_start(out=bm1_sb[:], in_=bm1c[:])
            bm2_sb = wS.tile([OUT, 1], F32, tag="bm2")
            nc.sync.dma_start(out=bm2_sb[:], in_=bm2c[:])

            # chunk 2 of the m0 AllGather: bank-0/1 gathers don't
            # depend on it and start draining immediately
            nc.gpsimd.collective_compute(
                "AllGather", mybir.AluOpType.bypass,
                replica_groups=[list(range(NCORES))],
                ins=[m0_loc.ap()[R1:NPAD, :].opt()],
                outs=[m0_full.ap()[NCORES * R1:NPHYS, :].opt()],
            )

            gt = {}

            def agg_block(g, bi, b, mloc, layer):
                nb = int(n_b[b])
                t0 = int(chunk_base_b[b])
                eq = eP.tile([128, NBMAX, 128], BF16, tag="eq", name="eq")
                nc.vector.tensor_tensor(
                    out=eq[:, :nb, :], in0=iota_sb[:, :nb, :],
                    in1=dstL_sb[:, t0:t0 + nb].unsqueeze(2)
                    .broadcast_to([128, nb, 128]),
                    op=mybir.AluOpType.is_equal)
                acc = psAcc.tile([HID, 128], F32, space="PSUM")
                # self-loop: acc[h, d] += table[d, h]
                nc.tensor.matmul(
                    out=acc[:], lhsT=mloc[:, bi, :], rhs=ident_sb[:],
                    start=True, stop=False)
                mm = 0
                last = (layer == 2)
                for q in range(NBANK):
                    for j in range(int(nch_bq[b, q])):
                        mm += 1
                        nc.tensor.matmul(
                            out=acc[:],
                            lhsT=gt[layer, g, q][:, int(seg_choff[b, q])
                                                 + j, :],
                            rhs=eq[:, int(qoff_bq[b, q]) + j, :],
                            start=False, stop=(last and mm == nb),
                        )
                return acc

            # ---------------- layer-1 sweep ----------------
            slab = [None]
            for g in range(NG):
                nblk = len(list(_group_blocks(g)))
                mloc = mlp.tile([128, G, HID], BF16, tag="mloc",
                                name="mloc")
                r0g = g * G * 128
                nc.sync.dma_start(
                    out=mloc[:, :nblk, :],
                    in_=m0_loc.ap()[r0g:r0g + nblk * 128, :]
                    .rearrange("(j p) f -> p j f", j=nblk))
                rdg = wG.tile([1, G * 128], BF16, tag="rdg", name="rdg")
                nc.sync.dma_start(out=rdg[:, :nblk * 128],
                                  in_=rdinvR[:, r0g:r0g + nblk * 128])
                for q in range(NBANK):
                    gt[1, g, q] = gather_instr(gqp, idx_sb, m0_full, g, q)
                for bi, b in enumerate(_group_blocks(g)):
                    acc = agg_block(g, bi, b, mloc, 1)
                    # += b1 (x) rdinv closes the accumulation
                    nc.tensor.matmul(
                        out=acc[:], lhsT=b1r_sb[:],
                        rhs=rdg[:, bi * 128:(bi + 1) * 128],
                        start=False, stop=True)
                    v = hS.tile([HID, 128], BF16, tag="v", name="v")
                    nc.scalar.activation(
                        out=v[:], in_=acc[:],
                        func=mybir.ActivationFunctionType.Copy)
                    m1ps = psM.tile([128, HID], F32, space="PSUM")
                    nc.tensor.matmul(out=m1ps[:], lhsT=v[:], rhs=w2_sb[:],
                                     start=True, stop=True)
                    jj = b % 4
                    if jj == 0:
                        slab[0] = hS.tile([128, 4, HID], BF16,
                                          tag="m1slab", name="m1slab")
                    nc.scalar.activation(
                        out=slab[0][:, jj, :], in_=m1ps[:],
                        func=mybir.ActivationFunctionType.Copy,
                        scale=dinv2C_sb[:, b:b + 1])
                    if jj == 3 or b == NB - 1:
                        rows = (jj + 1) * 128
                        base2 = (b - jj) * 128
                        nc.sync.dma_start(
                            out=m1_loc.ap()[base2:base2 + rows, :]
                            .rearrange("(j p) f -> p j f", j=jj + 1),
                            in_=slab[0][:, :jj + 1, :])
                    if (jj == 3 or b == NB - 1) and \
                            (b + 1) * 128 >= R1 > (b - 3) * 128:
                        # m1 rows [0, R1) stored: ship chunk 1 now
                        nc.gpsimd.collective_compute(
                            "AllGather", mybir.AluOpType.bypass,
                            replica_groups=[list(range(NCORES))],
                            ins=[m1_loc.ap()[0:R1, :].opt()],
                            outs=[m1_full.ap()[0:NCORES * R1, :].opt()],
                        )
                    if b == NB - 1:
                        nc.gpsimd.collective_compute(
                            "AllGather", mybir.AluOpType.bypass,
                            replica_groups=[list(range(NCORES))],
                            ins=[m1_loc.ap()[R1:NPAD, :].opt()],
                            outs=[m1_full.ap()
                                  [NCORES * R1:NPHYS, :].opt()],
                        )

            # ---------------- layer-2 sweep + MLP head ----------------
            for g in range(NG):
                nblk = len(list(_group_blocks(g)))
                mloc = mlp.tile([128, G, HID], BF16, tag="mloc",
                                name="mloc")
                r0g = g * G * 128
                nc.sync.dma_start(
                    out=mloc[:, :nblk, :],
                    in_=m1_loc.ap()[r0g:r0g + nblk * 128, :]
                    .rearrange("(j p) f -> p j f", j=nblk))
                dvg = wG.tile([128, G * 128], BF16, tag="dvg", name="dvg")
                nc.sync.dma_start(out=dvg[:, :nblk * 128],
                                  in_=dinvBT[:, r0g:r0g + nblk * 128])
                for q in range(NBANK):
                    gt[2, g, q] = gather_instr(gqp, idx_sb, m1_full, g, q)
                zslab = slabp.tile([OUT, G, 128], F32, tag="zslab",
                                   name="zslab")
                for bi, b in enumerate(_group_blocks(g)):
                    acc = agg_block(g, bi, b, mloc, 2)
                    # w = dinv * acc ; r2T = Relu(w + b2)
                    w = hS.tile([HID, 128], BF16, tag="w", name="w")
                    nc.vector.scalar_tensor_tensor(
                        out=w[:], in0=acc[:], scalar=0.0,
                        in1=dvg[:, bi * 128:(bi + 1) * 128],
                        op0=mybir.AluOpType.add,
                        op1=mybir.AluOpType.mult)
                    r2T = hS.tile([HID, 128], BF16, tag="r2T", name="r2T")
                    nc.scalar.activation(
                        out=r2T[:], in_=w[:],
                        func=mybir.ActivationFunctionType.Relu,
                        bias=b2_sb[:])
                    y1 = psY.tile([MID, 128], F32, space="PSUM")
                    nc.tensor.matmul(out=y1[:], lhsT=wm1_sb[:],
                                     rhs=r2T[:], start=True, stop=True)
                    r1T = hS.tile([MID, 128], BF16, tag="r1T", name="r1T")
                    nc.scalar.activation(
                        out=r1T[:], in_=y1[:],
                        func=mybir.ActivationFunctionType.Relu,
                        bias=bm1_sb[:])
                    z = psZ.tile([OUT, 128], F32, space="PSUM")
                    nc.tensor.matmul(out=z[:], lhsT=wm2_sb[:], rhs=r1T[:],
                                     start=True, stop=True)
                    nc.vector.tensor_scalar(
                        out=zslab[:, bi, :], in0=z[:], scalar1=bm2_sb[:],
                        scalar2=None, op0=mybir.AluOpType.add)
                c0 = g * G * 128
                cols = nblk * 128
                nc.sync.dma_start(
                    out=outT.ap()[:, c0:c0 + cols].rearrange(
                        "o (j p) -> o j p", j=nblk),
                    in_=zslab[:, :nblk, :])

    nc.compile()
    return nc


def make_inmaps(plan, inputs):
    bf = mybir.dt.np(BF16)
    x = np.asarray(inputs["x"], dtype=np.float32)
    w1 = np.asarray(inputs["w1"], np.float32).astype(bf)
    w2 = np.asarray(inputs["w2"], np.float32).astype(bf)
    wm1 = np.asarray(inputs["wm1"], np.float32).astype(bf)
    wm2 = np.asarray(inputs["wm2"], np.float32).astype(bf)
    b1r = np.asarray(inputs["b1"], np.float32).astype(bf)[None, :]
    b2c = np.asarray(inputs["b2"], np.float32)[:, None]
    bm1c = np.asarray(inputs["bm1"], np.float32)[:, None]
    bm2c = np.asarray(inputs["bm2"], np.float32)[:, None]
    in_maps = []
    for c in range(NCORES):
        xTc = np.zeros((IN_DIM, NPAD), dtype=bf)
        xTc[:, :NPC] = x[c * NPC:(c + 1) * NPC].T.astype(bf)
        in_maps.append({
            "xT": xTc, "idx16": plan["idx16"][c],
            "dstL": plan["dstL"][c], "iotaB": plan["iotaB"],
            "ident": plan["ident"],
            "w1": w1, "w2": w2, "wm1": wm1, "wm2": wm2,
            "b1r": b1r, "b2c": b2c, "bm1c": bm1c, "bm2c": bm2c,
            "dinv2C": plan["dinv2C"][c],
            "dinvBT": plan["dinvBT"][c], "rdinvR": plan["rdinvR"][c],
        })
    return in_maps


def build(inputs):
    plan = make_plan(np.asarray(inputs["edge_index"]))
    nc = build_gcn(plan)
    in_maps = make_inmaps(plan, inputs)
    return nc, in_maps


def assemble(results, cfg=None):
    return np.concatenate(
        [np.asarray(results[c]["outT"], dtype=np.float32).T[:NPC]
         for c in range(NCORES)], axis=0)


def kernel(**inputs):
    """Full-input entry point: returns [N, 4] float32."""
    nc, in_maps = build(inputs)
    from concourse.bass_utils import run_bass_kernel_spmd
    res = run_bass_kernel_spmd(nc, in_maps, core_ids=list(range(NCORES)))
    return assemble(res.results)
